# revision 8
# baseline (speedup 1.0000x reference)
"""Causal self-attention (B=4, T=2048, C=1024, H=16, D=64) on 8 TRN2 NeuronCores.

Sharding: core c -> (batch b = c//2, head-group g = c%2 covering heads
8g..8g+8). Data-parallel over B, tensor-parallel over heads. The output
projection is computed per-core over its 512 channels; the two partial
products per batch are summed on the host (the "all-reduce"), where the
projection bias is also added.

Per-core kernel (single SPMD program, per-core data):
  - qT = (x Wq + bq)^T and kT likewise, laid out [c'=512, T] (head-major on
    partitions: chunk m holds heads 2m, 2m+1 at partition offsets 0/64); v in
    natural layout [T, c'] with an appended ones column per head (Vaug, M=65)
    for the softmax denominator.
  - attention per head pair m, per 512-wide t-block: S^T[s,t] tiles via K=64
    matmuls packed two-heads-per-PE-array (tile_position derives from
    base_partition 0/64); exp on ScalarE (scale=1/8 folded) straight from PSUM
    into bf16 SBUF; causal masking by multiplying constant triangular masks on
    the diagonal s-chunks; AV matmuls with lhsT=Vaug -> unnormalized y^T (rows
    0:64) and sumexp (row 64) in one PSUM accumulation; normalize with
    partition_broadcast + fast reciprocal.
  - y_out^T = Wp_g^T @ yT accumulated over the core's 4 channel chunks,
    streamed to DRAM as bf16.

QKV compute, attention, and the projection are interleaved so the ScalarE
(exp) stream starts early and PE work hides under it. After Tile scheduling,
redundant same-engine semaphore waits are stripped so cross-engine waits ride
the instructions themselves instead of spawning EVENT_SEMAPHORE ops on the
busy Scalar queue.

No max-subtraction in softmax: scores are O(1) here (exp is safe in fp32),
and exp(S)/sum(exp(S)) is mathematically identical to jax.nn.softmax.
"""

import numpy as np
import ml_dtypes

BF16 = ml_dtypes.bfloat16
F32 = np.float32

N_EMBD = 1024
N_HEAD = 16
HEAD_DIM = 64
B = 4
T = 2048
N_CORES = 8
HPC = 8          # heads per core
CPC = HPC * HEAD_DIM  # channels per core = 512
NKC = N_EMBD // 128   # contraction chunks over full embed = 8
NM = CPC // 128       # head-pair chunks per core = 4
NB = T // 512         # 512-wide t blocks = 4
NTT = T // 128        # 128-wide t tiles = 16

_BUILT = {}
LAST_RESULT = None  # BassKernelResults of the most recent run (for test harness)


def _strip_self_waits(nc, mybir):
    """Remove same-engine semaphore waits (vacuous on an in-order queue for
    equal-shape streaming ops) so the single HW wait slot is free for the
    real cross-engine dependency."""
    pfx = {
        mybir.EngineType.Activation: "Activation_",
        mybir.EngineType.DVE: "DVE_",
        mybir.EngineType.PE: "PE_",
        mybir.EngineType.Pool: "Pool_",
        mybir.EngineType.SP: "SP_",
    }
    n = 0
    for blk in nc.main_func.blocks:
        for ins in blk.instructions:
            si = ins.sync_info
            if si is None or not si.on_wait:
                continue
            p = pfx.get(ins.engine)
            if not p:
                continue
            kept = [w for w in si.on_wait
                    if not (w.ant_name and w.ant_name.startswith(p)
                            and w.wait_mode == "sem-ge-imm")]
            if len(kept) != len(si.on_wait):
                n += len(si.on_wait) - len(kept)
                si.on_wait = kept
    return n


def _build_nc():
    import concourse.bass as bass
    import concourse.mybir as mybir
    import concourse.tile as tile
    from concourse import bacc

    dt = mybir.dt
    AF = mybir.ActivationFunctionType
    ALU = mybir.AluOpType

    nc = bacc.Bacc(trn_type="TRN2", name="csa")

    # ---- DRAM I/O ----
    xT_d = nc.dram_tensor("xT", [N_EMBD, T], dt.bfloat16, kind="ExternalInput")
    wq_d = nc.dram_tensor("wq", [N_EMBD, CPC], dt.bfloat16, kind="ExternalInput")
    wk_d = nc.dram_tensor("wk", [N_EMBD, CPC], dt.bfloat16, kind="ExternalInput")
    wv_d = nc.dram_tensor("wv", [N_EMBD, CPC], dt.bfloat16, kind="ExternalInput")
    wp_d = nc.dram_tensor("wp", [CPC, N_EMBD], dt.bfloat16, kind="ExternalInput")
    bq_d = nc.dram_tensor("bq_r", [128, NM], dt.float32, kind="ExternalInput")
    bk_d = nc.dram_tensor("bk_r", [128, NM], dt.float32, kind="ExternalInput")
    bv_d = nc.dram_tensor("bv_bc", [128, CPC], dt.float32, kind="ExternalInput")
    mk_d = nc.dram_tensor("masks", [128, 4, 512], dt.bfloat16, kind="ExternalInput")

    kT_o = nc.dram_tensor("kT_out", [CPC, T], dt.bfloat16, kind="ExternalOutput")
    v_o = nc.dram_tensor("v_out", [T, CPC], dt.bfloat16, kind="ExternalOutput")
    yp_o = nc.dram_tensor("ypT_out", [N_EMBD, T], dt.bfloat16, kind="ExternalOutput")

    with tile.TileContext(nc) as tc:
        with tc.tile_pool(name="const", bufs=1) as cp, \
             tc.tile_pool(name="ex", bufs=6) as exp_pool, \
             tc.tile_pool(name="stage", bufs=4) as stage, \
             tc.tile_pool(name="small", bufs=4) as small:

            # ---- load inputs, ordered so compute can start early ----
            wq_sb = cp.tile([128, NKC, CPC], dt.bfloat16, tag="wq")
            nc.sync.dma_start(wq_sb, wq_d.ap().rearrange("(kc p) m -> p kc m", p=128))
            xT_sb = cp.tile([128, NKC, T], dt.bfloat16, tag="xT")
            xT_r = xT_d.ap().rearrange("(kc p) t -> p kc t", p=128)
            for bb in range(NB):
                sl = slice(512 * bb, 512 * bb + 512)
                nc.sync.dma_start(xT_sb[:, :, sl], xT_r[:, :, sl])
            bq_sb = cp.tile([128, NM], dt.float32, tag="bq")
            nc.sync.dma_start(bq_sb, bq_d.ap())
            wk_sb = cp.tile([128, NKC, CPC], dt.bfloat16, tag="wk")
            nc.sync.dma_start(wk_sb, wk_d.ap().rearrange("(kc p) m -> p kc m", p=128))
            bk_sb = cp.tile([128, NM], dt.float32, tag="bk")
            nc.sync.dma_start(bk_sb, bk_d.ap())
            mk_sb = cp.tile([128, 4, 512], dt.bfloat16, tag="mk")
            nc.sync.dma_start(mk_sb, mk_d.ap())
            wv_sb = cp.tile([128, NKC, CPC], dt.bfloat16, tag="wv")
            nc.sync.dma_start(wv_sb, wv_d.ap().rearrange("(kc p) m -> p kc m", p=128))
            bv_sb = cp.tile([128, CPC], dt.float32, tag="bv")
            nc.sync.dma_start(bv_sb, bv_d.ap())
            wp_sb = cp.tile([128, NM, N_EMBD], dt.bfloat16, tag="wp")
            nc.sync.dma_start(wp_sb, wp_d.ap().rearrange("(kc p) m -> p kc m", p=128))

            qT_sb = cp.tile([128, NM, T], dt.bfloat16, tag="qT")
            kT_sb = cp.tile([128, NM, T], dt.bfloat16, tag="kT")
            # Vaug: [t%128, tt, head, 65] (col 64 = ones)
            v_sb = cp.tile([128, NTT, HPC, 65], dt.bfloat16, tag="v")
            yT_sb = cp.tile([128, NM, T], dt.bfloat16, tag="yT")

            nc.gpsimd.memset(v_sb[:, :, :, 64:65], 1.0)

            kT_o_r = kT_o.ap().rearrange("(m p) t -> p m t", p=128)
            v_o_r = v_o.ap().rearrange("(tt p) (h d) -> p tt h d", p=128, h=HPC)
            yp_r = yp_o.ap().rearrange("(mo p) t -> p mo t", p=128)

            # PSUM pools: sps + avps live throughout; qkps (QKV) is swapped
            # for pps (projection) once the last QK pair is emitted.
            sps_cm = tc.tile_pool(name="sps", bufs=2, space="PSUM")
            sps = sps_cm.__enter__()
            avps_cm = tc.tile_pool(name="avps", bufs=1, space="PSUM")
            avps = avps_cm.__enter__()
            qkps_cm = tc.tile_pool(name="qkps", bufs=2, space="PSUM")
            qkps = qkps_cm.__enter__()

            def qk_pair(m):
                for (w_sb, bias_sb, out_sb, nm) in ((wq_sb, bq_sb, qT_sb, "q"),
                                                    (wk_sb, bk_sb, kT_sb, "k")):
                    for bb in range(NB):
                        ps = qkps.tile([128, 512], dt.float32, tag="qk",
                                       name=f"{nm}_{m}_{bb}")
                        for kc in range(NKC):
                            nc.tensor.matmul(
                                ps,
                                lhsT=w_sb[:, kc, 128 * m:128 * m + 128],
                                rhs=xT_sb[:, kc, 512 * bb:512 * bb + 512],
                                start=(kc == 0), stop=(kc == NKC - 1))
                        nc.vector.tensor_scalar(
                            out=out_sb[:, m, 512 * bb:512 * bb + 512],
                            in0=ps, scalar1=bias_sb[:, m:m + 1], scalar2=None,
                            op0=ALU.add)
                nc.sync.dma_start(kT_o_r[:, m], kT_sb[:, m])

            def v_tt(tt):
                ps = qkps.tile([128, 512], dt.float32, tag="qk", name=f"v_{tt}")
                for kc in range(NKC):
                    nc.tensor.matmul(
                        ps,
                        lhsT=xT_sb[:, kc, 128 * tt:128 * tt + 128],
                        rhs=wv_sb[:, kc, :],
                        start=(kc == 0), stop=(kc == NKC - 1))
                nc.vector.tensor_tensor(
                    out=v_sb[:, tt, :, 0:64],
                    in0=ps.rearrange("p (h d) -> p h d", h=HPC),
                    in1=bv_sb.rearrange("p (h d) -> p h d", h=HPC),
                    op=ALU.add)
                nc.sync.dma_start(v_o_r[:, tt], v_sb[:, tt, :, 0:64])

            def attn_block(m, bb):
                njc = 4 * bb + 4  # causal s-chunk count for this t block
                ps_y = [avps.tile([65, 512], dt.float32, tag=f"av{h}",
                                  name=f"av_{h}_{m}_{bb}")
                        for h in range(2)]
                for g in range(njc // 2):
                    for half in range(2):
                        p0 = 64 * half
                        ps_s = sps.tile([128, 2, 512], dt.float32, tag="s",
                                        name=f"s_{m}_{bb}_{g}_{half}")
                        for dj in range(2):
                            j = 2 * g + dj
                            nc.tensor.matmul(
                                ps_s[:, dj, :],
                                lhsT=kT_sb[p0:p0 + 64, m,
                                           128 * j:128 * j + 128],
                                rhs=qT_sb[p0:p0 + 64, m,
                                          512 * bb:512 * bb + 512],
                                start=True, stop=True)
                        ex = exp_pool.tile([128, 2, 512], dt.bfloat16,
                                           tag="ex", name=f"ex_{m}_{bb}_{g}_{half}")
                        nc.scalar.activation(ex, ps_s, AF.Exp, scale=0.125)
                        for dj in range(2):
                            j = 2 * g + dj
                            jpos = j - 4 * bb
                            if jpos >= 0:  # diagonal chunk: causal mask
                                nc.vector.tensor_tensor(
                                    ex[:, dj, :], ex[:, dj, :],
                                    mk_sb[:, jpos, :], ALU.mult)
                        for dj in range(2):
                            j = 2 * g + dj
                            nc.tensor.matmul(
                                ps_y[half],
                                lhsT=v_sb[:, j, 2 * m + half, :],
                                rhs=ex[:, dj, :],
                                start=(j == 0), stop=(j == njc - 1))
                for half in range(2):
                    row = small.tile([1, 512], dt.float32, tag="row",
                                     name=f"row_{m}_{bb}_{half}")
                    nc.vector.tensor_copy(row, ps_y[half][64:65, :])
                    bc = small.tile([64, 512], dt.float32, tag="bc",
                                    name=f"bc_{m}_{bb}_{half}")
                    nc.gpsimd.partition_broadcast(bc, row, channels=64)
                    rec = small.tile([64, 512], dt.float32, tag="rec",
                                     name=f"rec_{m}_{bb}_{half}")
                    nc.vector.reciprocal_approx_fast(out=rec, in_=bc)
                    nc.vector.tensor_tensor(
                        yT_sb[64 * half:64 * half + 64, m,
                              512 * bb:512 * bb + 512],
                        ps_y[half][0:64, :], rec, ALU.mult)

            # emission order = scheduling priority: start attention ASAP,
            # hide remaining QKV + projection under the ScalarE exp stream.
            qk_pair(0)
            for tt in range(4):
                v_tt(tt)
            attn_block(0, 0)
            qk_pair(1)
            for tt in range(4, 8):
                v_tt(tt)
            attn_block(0, 1)
            for tt in range(8, 12):
                v_tt(tt)
            attn_block(0, 2)
            for tt in range(12, 16):
                v_tt(tt)
            qk_pair(2)
            attn_block(0, 3)
            attn_block(1, 0)
            qk_pair(3)
            attn_block(1, 1)
            attn_block(1, 2)
            attn_block(1, 3)

            # QKV psum pool -> projection psum pool
            qkps_cm.__exit__(None, None, None)
            pps_cm = tc.tile_pool(name="pps", bufs=2, space="PSUM")
            pps = pps_cm.__enter__()

            def proj_bb(bb):
                for mo in range(N_EMBD // 128):
                    ps = pps.tile([128, 512], dt.float32, tag="pp",
                                  name=f"pp_{mo}_{bb}")
                    for kc in range(NM):
                        nc.tensor.matmul(
                            ps,
                            lhsT=wp_sb[:, kc, 128 * mo:128 * mo + 128],
                            rhs=yT_sb[:, kc, 512 * bb:512 * bb + 512],
                            start=(kc == 0), stop=(kc == NM - 1))
                    st = stage.tile([128, 512], dt.bfloat16, tag="st",
                                    name=f"st_{mo}_{bb}")
                    nc.vector.tensor_copy(st, ps)
                    nc.sync.dma_start(yp_r[:, mo, 512 * bb:512 * bb + 512], st)

            attn_block(2, 0)
            attn_block(2, 1)
            attn_block(2, 2)
            attn_block(2, 3)
            attn_block(3, 0)
            proj_bb(0)
            attn_block(3, 1)
            proj_bb(1)
            attn_block(3, 2)
            proj_bb(2)
            attn_block(3, 3)
            proj_bb(3)

            pps_cm.__exit__(None, None, None)
            avps_cm.__exit__(None, None, None)
            sps_cm.__exit__(None, None, None)

    _strip_self_waits(nc, mybir)
    nc.finalize()
    return nc


def _get_nc():
    if "nc" not in _BUILT:
        _BUILT["nc"] = _build_nc()
    return _BUILT["nc"]


def _make_masks():
    sp = np.arange(128)[:, None]
    tp = np.arange(512)[None, :]
    return np.stack([(tp >= 128 * jpos + sp) for jpos in range(4)],
                    axis=1).astype(BF16)  # [128, 4, 512]


def kernel(x, Wq, bq, Wk, bk, Wv, bv, Wp, bp):
    global LAST_RESULT
    from concourse.bass_utils import run_bass_kernel_spmd

    x = np.asarray(x, F32)
    Wq = np.asarray(Wq, F32); bq = np.asarray(bq, F32)
    Wk = np.asarray(Wk, F32); bk = np.asarray(bk, F32)
    Wv = np.asarray(Wv, F32); bv = np.asarray(bv, F32)
    Wp = np.asarray(Wp, F32); bp = np.asarray(bp, F32)

    nc = _get_nc()
    masks = _make_masks()
    xT = np.ascontiguousarray(x.transpose(0, 2, 1))  # [B, C, T]

    in_maps = []
    for c in range(N_CORES):
        b, g = divmod(c, 2)
        sl = slice(CPC * g, CPC * g + CPC)
        in_maps.append({
            "xT": xT[b].astype(BF16),
            "wq": Wq[:, sl].astype(BF16),
            "wk": Wk[:, sl].astype(BF16),
            "wv": Wv[:, sl].astype(BF16),
            "wp": Wp[sl, :].astype(BF16),
            "bq_r": np.ascontiguousarray(bq[sl].reshape(NM, 128).T),
            "bk_r": np.ascontiguousarray(bk[sl].reshape(NM, 128).T),
            "bv_bc": np.ascontiguousarray(
                np.broadcast_to(bv[sl], (128, CPC))).astype(F32),
            "masks": masks,
        })

    res = run_bass_kernel_spmd(nc, in_maps, core_ids=list(range(N_CORES)))
    LAST_RESULT = res

    y = np.empty((B, T, N_EMBD), F32)
    k = np.empty((B, N_HEAD, T, HEAD_DIM), F32)
    v = np.empty((B, N_HEAD, T, HEAD_DIM), F32)
    for c in range(N_CORES):
        b, g = divmod(c, 2)
        out = res.results[c]
        kT = out["kT_out"].astype(F32)           # [512, T]
        vn = out["v_out"].astype(F32)            # [T, 512]
        for lh in range(HPC):
            h = HPC * g + lh
            k[b, h] = kT[64 * lh:64 * lh + 64, :].T
            v[b, h] = vn[:, 64 * lh:64 * lh + 64]
    for b in range(B):
        ypT = (res.results[2 * b]["ypT_out"].astype(F32)
               + res.results[2 * b + 1]["ypT_out"].astype(F32))  # [C, T]
        y[b] = ypT.T + bp[None, :]

    present = np.stack([k, v])  # [2, B, H, T, D]
    return y, present


# revision 14
# speedup vs baseline: 1.0072x; 1.0072x over previous
"""Causal self-attention (B=4, T=2048, C=1024, H=16, D=64) on 8 TRN2 NeuronCores.

Sharding: core c -> (batch b = c//2, head-group g = c%2 covering heads
8g..8g+8). Data-parallel over B, tensor-parallel over heads. The output
projection is computed per-core over its 512 channels; the two partial
products per batch are summed on the host (the "all-reduce"), where the
projection bias is also added.

Per-core kernel (single SPMD program, per-core data):
  - qT = (x Wq + bq)^T and kT likewise, laid out [c'=512, T] (head-major on
    partitions: chunk m holds heads 2m, 2m+1 at partition offsets 0/64); v in
    natural layout [T, c'] with an appended ones column per head (Vaug, M=65)
    for the softmax denominator.
  - attention per head pair m, per 512-wide t-block: S^T[s,t] tiles via K=64
    matmuls packed two-heads-per-PE-array (tile_position derives from
    base_partition 0/64); exp on ScalarE (scale=1/8 folded) straight from PSUM
    into bf16 SBUF; causal masking by multiplying constant triangular masks on
    the diagonal s-chunks; AV matmuls with lhsT=Vaug -> unnormalized y^T (rows
    0:64) and sumexp (row 64) in one PSUM accumulation; normalize with
    partition_broadcast + fast reciprocal.
  - y_out^T = Wp_g^T @ yT accumulated over the core's 4 channel chunks,
    streamed to DRAM as bf16.

QKV compute, attention, and the projection are interleaved so the ScalarE
(exp) stream starts early and PE work hides under it. After Tile scheduling,
redundant same-engine semaphore waits are stripped so cross-engine waits ride
the instructions themselves instead of spawning EVENT_SEMAPHORE ops on the
busy Scalar queue.

No max-subtraction in softmax: scores are O(1) here (exp is safe in fp32),
and exp(S)/sum(exp(S)) is mathematically identical to jax.nn.softmax.
"""

import numpy as np
import ml_dtypes

BF16 = ml_dtypes.bfloat16
F32 = np.float32

N_EMBD = 1024
N_HEAD = 16
HEAD_DIM = 64
B = 4
T = 2048
N_CORES = 8
HPC = 8          # heads per core
CPC = HPC * HEAD_DIM  # channels per core = 512
NKC = N_EMBD // 128   # contraction chunks over full embed = 8
NM = CPC // 128       # head-pair chunks per core = 4
NB = T // 512         # 512-wide t blocks = 4
NTT = T // 128        # 128-wide t tiles = 16

_BUILT = {}
LAST_RESULT = None  # BassKernelResults of the most recent run (for test harness)


def _strip_self_waits(nc, mybir):
    """Remove same-engine semaphore waits (vacuous on an in-order queue for
    equal-shape streaming ops) so the single HW wait slot is free for the
    real cross-engine dependency."""
    pfx = {
        mybir.EngineType.Activation: "Activation_",
        mybir.EngineType.DVE: "DVE_",
        mybir.EngineType.PE: "PE_",
        mybir.EngineType.Pool: "Pool_",
        mybir.EngineType.SP: "SP_",
    }
    n = 0
    for blk in nc.main_func.blocks:
        for ins in blk.instructions:
            si = ins.sync_info
            if si is None or not si.on_wait:
                continue
            p = pfx.get(ins.engine)
            if not p:
                continue
            kept = [w for w in si.on_wait
                    if not (w.ant_name and w.ant_name.startswith(p)
                            and w.wait_mode == "sem-ge-imm")]
            if len(kept) != len(si.on_wait):
                n += len(si.on_wait) - len(kept)
                si.on_wait = kept
    return n


def _build_nc():
    import concourse.bass as bass
    import concourse.mybir as mybir
    import concourse.tile as tile
    from concourse import bacc

    dt = mybir.dt
    AF = mybir.ActivationFunctionType
    ALU = mybir.AluOpType

    nc = bacc.Bacc(trn_type="TRN2", name="csa")

    # ---- DRAM I/O (host pre-arranges for contiguous per-partition DMA) ----
    xT_d = nc.dram_tensor("xT", [NB, 128, NKC, 512], dt.bfloat16,
                          kind="ExternalInput")
    wq_d = nc.dram_tensor("wq", [128, NKC, CPC], dt.bfloat16, kind="ExternalInput")
    wk_d = nc.dram_tensor("wk", [128, NKC, CPC], dt.bfloat16, kind="ExternalInput")
    wv_d = nc.dram_tensor("wv", [128, NKC, CPC], dt.bfloat16, kind="ExternalInput")
    wp_d = nc.dram_tensor("wp", [128, NM, N_EMBD], dt.bfloat16,
                          kind="ExternalInput")
    bq_d = nc.dram_tensor("bq_r", [128, NM], dt.float32, kind="ExternalInput")
    bk_d = nc.dram_tensor("bk_r", [128, NM], dt.float32, kind="ExternalInput")
    bv_d = nc.dram_tensor("bv_bc", [128, CPC], dt.float32, kind="ExternalInput")
    mk_d = nc.dram_tensor("masks", [128, 4, 512], dt.bfloat16, kind="ExternalInput")

    kT_o = nc.dram_tensor("kT_out", [CPC, T], dt.bfloat16, kind="ExternalOutput")
    v_o = nc.dram_tensor("v_out", [T, CPC], dt.bfloat16, kind="ExternalOutput")
    yp_o = nc.dram_tensor("ypT_out", [N_EMBD, T], dt.bfloat16, kind="ExternalOutput")

    with tile.TileContext(nc) as tc:
        with tc.tile_pool(name="const", bufs=1) as cp, \
             tc.tile_pool(name="ex", bufs=6) as exp_pool, \
             tc.tile_pool(name="stage", bufs=4) as stage, \
             tc.tile_pool(name="small", bufs=4) as small:

            # ---- load inputs, ordered so compute can start early ----
            wq_sb = cp.tile([128, NKC, CPC], dt.bfloat16, tag="wq")
            nc.sync.dma_start(wq_sb, wq_d.ap())
            # x^T, bb-major: [p, bb, kc, 512]
            xT_sb = cp.tile([128, NB, NKC, 512], dt.bfloat16, tag="xT")
            for bb in range(NB):
                nc.sync.dma_start(xT_sb[:, bb], xT_d.ap()[bb])
            bq_sb = cp.tile([128, NM], dt.float32, tag="bq")
            nc.sync.dma_start(bq_sb, bq_d.ap())
            wk_sb = cp.tile([128, NKC, CPC], dt.bfloat16, tag="wk")
            nc.sync.dma_start(wk_sb, wk_d.ap())
            bk_sb = cp.tile([128, NM], dt.float32, tag="bk")
            nc.sync.dma_start(bk_sb, bk_d.ap())
            mk_sb = cp.tile([128, 4, 512], dt.bfloat16, tag="mk")
            nc.sync.dma_start(mk_sb, mk_d.ap())
            wv_sb = cp.tile([128, NKC, CPC], dt.bfloat16, tag="wv")
            nc.sync.dma_start(wv_sb, wv_d.ap())
            bv_sb = cp.tile([128, CPC], dt.float32, tag="bv")
            nc.sync.dma_start(bv_sb, bv_d.ap())
            wp_sb = cp.tile([128, NM, N_EMBD], dt.bfloat16, tag="wp")
            nc.sync.dma_start(wp_sb, wp_d.ap())

            qT_sb = cp.tile([128, NM, T], dt.bfloat16, tag="qT")
            kT_sb = cp.tile([128, NM, T], dt.bfloat16, tag="kT")
            # Vaug: [t%128, tt, head, 65] (col 64 = ones)
            v_sb = cp.tile([128, NTT, HPC, 65], dt.bfloat16, tag="v")
            yT_sb = cp.tile([128, NM, T], dt.bfloat16, tag="yT")

            nc.gpsimd.memset(v_sb[:, :, :, 64:65], 1.0)

            kT_o_r = kT_o.ap().rearrange("(m p) t -> p m t", p=128)
            v_o_r = v_o.ap().rearrange("(tt p) (h d) -> p tt h d", p=128, h=HPC)
            yp_r = yp_o.ap().rearrange("(mo p) t -> p mo t", p=128)

            # PSUM pools: sps + avps live throughout; qkps (QKV) is swapped
            # for pps (projection) once the last QK pair is emitted.
            sps_cm = tc.tile_pool(name="sps", bufs=2, space="PSUM")
            sps = sps_cm.__enter__()
            avps_cm = tc.tile_pool(name="avps", bufs=1, space="PSUM")
            avps = avps_cm.__enter__()
            qkps_cm = tc.tile_pool(name="qkps", bufs=2, space="PSUM")
            qkps = qkps_cm.__enter__()

            def qk_pair(m):
                for (w_sb, bias_sb, out_sb, nm) in ((wq_sb, bq_sb, qT_sb, "q"),
                                                    (wk_sb, bk_sb, kT_sb, "k")):
                    for bb in range(NB):
                        ps = qkps.tile([128, 512], dt.float32, tag="qk",
                                       name=f"{nm}_{m}_{bb}")
                        for kc in range(NKC):
                            nc.tensor.matmul(
                                ps,
                                lhsT=w_sb[:, kc, 128 * m:128 * m + 128],
                                rhs=xT_sb[:, bb, kc, :],
                                start=(kc == 0), stop=(kc == NKC - 1))
                        nc.vector.tensor_scalar(
                            out=out_sb[:, m, 512 * bb:512 * bb + 512],
                            in0=ps, scalar1=bias_sb[:, m:m + 1], scalar2=None,
                            op0=ALU.add)
                nc.sync.dma_start(kT_o_r[:, m], kT_sb[:, m])

            def v_tt(tt):
                ps = qkps.tile([128, 512], dt.float32, tag="qk", name=f"v_{tt}")
                for kc in range(NKC):
                    nc.tensor.matmul(
                        ps,
                        lhsT=xT_sb[:, tt // 4, kc, 128 * (tt % 4):128 * (tt % 4) + 128],
                        rhs=wv_sb[:, kc, :],
                        start=(kc == 0), stop=(kc == NKC - 1))
                nc.vector.tensor_tensor(
                    out=v_sb[:, tt, :, 0:64],
                    in0=ps.rearrange("p (h d) -> p h d", h=HPC),
                    in1=bv_sb.rearrange("p (h d) -> p h d", h=HPC),
                    op=ALU.add)
                nc.sync.dma_start(v_o_r[:, tt], v_sb[:, tt, :, 0:64])

            def attn_block(m, bb):
                njc = 4 * bb + 4  # causal s-chunk count for this t block
                ps_y = [avps.tile([65, 512], dt.float32, tag=f"av{h}",
                                  name=f"av_{h}_{m}_{bb}")
                        for h in range(2)]
                for g in range(njc // 2):
                    # S matmuls for both heads interleaved so each j's two
                    # K=64 row-tiled matmuls (tile_position rows 0/64) sit
                    # adjacent in the PE stream and run concurrently.
                    ps_s = [sps.tile([128, 2, 512], dt.float32, tag="s",
                                     name=f"s_{m}_{bb}_{g}_{half}")
                            for half in range(2)]
                    for dj in range(2):
                        j = 2 * g + dj
                        for half in range(2):
                            p0 = 64 * half
                            nc.tensor.matmul(
                                ps_s[half][:, dj, :],
                                lhsT=kT_sb[p0:p0 + 64, m,
                                           128 * j:128 * j + 128],
                                rhs=qT_sb[p0:p0 + 64, m,
                                          512 * bb:512 * bb + 512],
                                start=True, stop=True)
                    exs = []
                    for half in range(2):
                        ex = exp_pool.tile([128, 2, 512], dt.bfloat16,
                                           tag="ex", name=f"ex_{m}_{bb}_{g}_{half}")
                        nc.scalar.activation(ex, ps_s[half], AF.Exp, scale=0.125)
                        exs.append(ex)
                    for half in range(2):
                        for dj in range(2):
                            j = 2 * g + dj
                            jpos = j - 4 * bb
                            if jpos >= 0:  # diagonal chunk: causal mask
                                nc.vector.tensor_tensor(
                                    exs[half][:, dj, :], exs[half][:, dj, :],
                                    mk_sb[:, jpos, :], ALU.mult)
                    for half in range(2):
                        for dj in range(2):
                            j = 2 * g + dj
                            nc.tensor.matmul(
                                ps_y[half],
                                lhsT=v_sb[:, j, 2 * m + half, :],
                                rhs=exs[half][:, dj, :],
                                start=(j == 0), stop=(j == njc - 1))
                for half in range(2):
                    row = small.tile([1, 512], dt.float32, tag="row",
                                     name=f"row_{m}_{bb}_{half}")
                    nc.vector.tensor_copy(row, ps_y[half][64:65, :])
                    bc = small.tile([64, 512], dt.float32, tag="bc",
                                    name=f"bc_{m}_{bb}_{half}")
                    nc.gpsimd.partition_broadcast(bc, row, channels=64)
                    rec = small.tile([64, 512], dt.float32, tag="rec",
                                     name=f"rec_{m}_{bb}_{half}")
                    nc.vector.reciprocal_approx_fast(out=rec, in_=bc)
                    nc.vector.tensor_tensor(
                        yT_sb[64 * half:64 * half + 64, m,
                              512 * bb:512 * bb + 512],
                        ps_y[half][0:64, :], rec, ALU.mult)

            # emission order = scheduling priority: start attention ASAP,
            # hide remaining QKV + projection under the ScalarE exp stream.
            qk_pair(0)
            for tt in range(4):
                v_tt(tt)
            attn_block(0, 0)
            qk_pair(1)
            for tt in range(4, 8):
                v_tt(tt)
            attn_block(0, 1)
            for tt in range(8, 12):
                v_tt(tt)
            attn_block(0, 2)
            for tt in range(12, 16):
                v_tt(tt)
            qk_pair(2)
            attn_block(0, 3)
            attn_block(1, 0)
            qk_pair(3)
            attn_block(1, 1)
            attn_block(1, 2)
            attn_block(1, 3)

            # QKV psum pool -> projection psum pool
            qkps_cm.__exit__(None, None, None)
            pps_cm = tc.tile_pool(name="pps", bufs=2, space="PSUM")
            pps = pps_cm.__enter__()

            def proj_bb(bb):
                for mo in range(N_EMBD // 128):
                    ps = pps.tile([128, 512], dt.float32, tag="pp",
                                  name=f"pp_{mo}_{bb}")
                    for kc in range(NM):
                        nc.tensor.matmul(
                            ps,
                            lhsT=wp_sb[:, kc, 128 * mo:128 * mo + 128],
                            rhs=yT_sb[:, kc, 512 * bb:512 * bb + 512],
                            start=(kc == 0), stop=(kc == NM - 1))
                    st = stage.tile([128, 512], dt.bfloat16, tag="st",
                                    name=f"st_{mo}_{bb}")
                    nc.vector.tensor_copy(st, ps)
                    nc.sync.dma_start(yp_r[:, mo, 512 * bb:512 * bb + 512], st)

            attn_block(2, 0)
            attn_block(2, 1)
            attn_block(2, 2)
            attn_block(2, 3)
            attn_block(3, 0)
            proj_bb(0)
            attn_block(3, 1)
            proj_bb(1)
            attn_block(3, 2)
            proj_bb(2)
            attn_block(3, 3)
            proj_bb(3)

            pps_cm.__exit__(None, None, None)
            avps_cm.__exit__(None, None, None)
            sps_cm.__exit__(None, None, None)

    _strip_self_waits(nc, mybir)
    nc.finalize()
    return nc


def _get_nc():
    if "nc" not in _BUILT:
        _BUILT["nc"] = _build_nc()
    return _BUILT["nc"]


def _make_masks():
    sp = np.arange(128)[:, None]
    tp = np.arange(512)[None, :]
    return np.stack([(tp >= 128 * jpos + sp) for jpos in range(4)],
                    axis=1).astype(BF16)  # [128, 4, 512]


def kernel(x, Wq, bq, Wk, bk, Wv, bv, Wp, bp):
    global LAST_RESULT
    from concourse.bass_utils import run_bass_kernel_spmd

    x = np.asarray(x, F32)
    Wq = np.asarray(Wq, F32); bq = np.asarray(bq, F32)
    Wk = np.asarray(Wk, F32); bk = np.asarray(bk, F32)
    Wv = np.asarray(Wv, F32); bv = np.asarray(bv, F32)
    Wp = np.asarray(Wp, F32); bp = np.asarray(bp, F32)

    nc = _get_nc()
    masks = _make_masks()
    xT = np.ascontiguousarray(x.transpose(0, 2, 1))  # [B, C, T]
    # [C, T] -> [NB, 128, NKC, 512]: (kc*128+p, bb*512+t') -> [bb, p, kc, t']
    xT_r = [np.ascontiguousarray(
        xT[b].reshape(NKC, 128, NB, 512).transpose(2, 1, 0, 3)).astype(BF16)
        for b in range(B)]

    def warr(w):  # [1024, 512] -> [128, NKC, 512]
        return np.ascontiguousarray(
            w.reshape(NKC, 128, CPC).transpose(1, 0, 2)).astype(BF16)

    in_maps = []
    for c in range(N_CORES):
        b, g = divmod(c, 2)
        sl = slice(CPC * g, CPC * g + CPC)
        in_maps.append({
            "xT": xT_r[b],
            "wq": warr(Wq[:, sl]),
            "wk": warr(Wk[:, sl]),
            "wv": warr(Wv[:, sl]),
            "wp": np.ascontiguousarray(
                Wp[sl, :].reshape(NM, 128, N_EMBD).transpose(1, 0, 2)).astype(BF16),
            "bq_r": np.ascontiguousarray(bq[sl].reshape(NM, 128).T),
            "bk_r": np.ascontiguousarray(bk[sl].reshape(NM, 128).T),
            "bv_bc": np.ascontiguousarray(
                np.broadcast_to(bv[sl], (128, CPC))).astype(F32),
            "masks": masks,
        })

    res = run_bass_kernel_spmd(nc, in_maps, core_ids=list(range(N_CORES)))
    LAST_RESULT = res

    y = np.empty((B, T, N_EMBD), F32)
    k = np.empty((B, N_HEAD, T, HEAD_DIM), F32)
    v = np.empty((B, N_HEAD, T, HEAD_DIM), F32)
    for c in range(N_CORES):
        b, g = divmod(c, 2)
        out = res.results[c]
        kT = out["kT_out"].astype(F32)           # [512, T]
        vn = out["v_out"].astype(F32)            # [T, 512]
        for lh in range(HPC):
            h = HPC * g + lh
            k[b, h] = kT[64 * lh:64 * lh + 64, :].T
            v[b, h] = vn[:, 64 * lh:64 * lh + 64]
    for b in range(B):
        ypT = (res.results[2 * b]["ypT_out"].astype(F32)
               + res.results[2 * b + 1]["ypT_out"].astype(F32))  # [C, T]
        y[b] = ypT.T + bp[None, :]

    present = np.stack([k, v])  # [2, B, H, T, D]
    return y, present


# revision 16
# speedup vs baseline: 1.0217x; 1.0144x over previous
"""Causal self-attention (B=4, T=2048, C=1024, H=16, D=64) on 8 TRN2 NeuronCores.

Sharding: core c -> (batch b = c//2, head-group g = c%2 covering heads
8g..8g+8). Data-parallel over B, tensor-parallel over heads. The output
projection is computed per-core over its 512 channels; the two partial
products per batch are summed on the host (the "all-reduce"), where the
projection bias is also added.

Per-core kernel (single SPMD program, per-core data):
  - qT = (x Wq + bq)^T and kT likewise, laid out [c'=512, T] (head-major on
    partitions: chunk m holds heads 2m, 2m+1 at partition offsets 0/64); v in
    natural layout [T, c'] with an appended ones column per head (Vaug, M=65)
    for the softmax denominator.
  - attention per head pair m, per 512-wide t-block: S^T[s,t] tiles via K=64
    matmuls packed two-heads-per-PE-array (tile_position derives from
    base_partition 0/64); exp on ScalarE (scale=1/8 folded) straight from PSUM
    into bf16 SBUF; causal masking by multiplying constant triangular masks on
    the diagonal s-chunks; AV matmuls with lhsT=Vaug -> unnormalized y^T (rows
    0:64) and sumexp (row 64) in one PSUM accumulation; normalize with
    partition_broadcast + fast reciprocal.
  - y_out^T = Wp_g^T @ yT accumulated over the core's 4 channel chunks,
    streamed to DRAM as bf16.

QKV compute, attention, and the projection are interleaved so the ScalarE
(exp) stream starts early and PE work hides under it. After Tile scheduling,
redundant same-engine semaphore waits are stripped so cross-engine waits ride
the instructions themselves instead of spawning EVENT_SEMAPHORE ops on the
busy Scalar queue.

No max-subtraction in softmax: scores are O(1) here (exp is safe in fp32),
and exp(S)/sum(exp(S)) is mathematically identical to jax.nn.softmax.
"""

import numpy as np
import ml_dtypes

BF16 = ml_dtypes.bfloat16
F32 = np.float32

N_EMBD = 1024
N_HEAD = 16
HEAD_DIM = 64
B = 4
T = 2048
N_CORES = 8
HPC = 8          # heads per core
CPC = HPC * HEAD_DIM  # channels per core = 512
NKC = N_EMBD // 128   # contraction chunks over full embed = 8
NM = CPC // 128       # head-pair chunks per core = 4
NB = T // 512         # 512-wide t blocks = 4
NTT = T // 128        # 128-wide t tiles = 16

_BUILT = {}
LAST_RESULT = None  # BassKernelResults of the most recent run (for test harness)


def _strip_self_waits(nc, mybir):
    """Remove same-engine semaphore waits (vacuous on an in-order queue for
    equal-shape streaming ops) so the single HW wait slot is free for the
    real cross-engine dependency."""
    pfx = {
        mybir.EngineType.Activation: "Activation_",
        mybir.EngineType.DVE: "DVE_",
        mybir.EngineType.PE: "PE_",
        mybir.EngineType.Pool: "Pool_",
        mybir.EngineType.SP: "SP_",
    }
    n = 0
    for blk in nc.main_func.blocks:
        for ins in blk.instructions:
            si = ins.sync_info
            if si is None or not si.on_wait:
                continue
            p = pfx.get(ins.engine)
            if not p:
                continue
            kept = [w for w in si.on_wait
                    if not (w.ant_name and w.ant_name.startswith(p)
                            and w.wait_mode == "sem-ge-imm")]
            if len(kept) != len(si.on_wait):
                n += len(si.on_wait) - len(kept)
                si.on_wait = kept
    return n


def _build_nc():
    import concourse.bass as bass
    import concourse.mybir as mybir
    import concourse.tile as tile
    from concourse import bacc

    dt = mybir.dt
    AF = mybir.ActivationFunctionType
    ALU = mybir.AluOpType

    nc = bacc.Bacc(trn_type="TRN2", name="csa")

    # ---- DRAM I/O (host pre-arranges for contiguous per-partition DMA) ----
    xT_d = nc.dram_tensor("xT", [NB, 128, NKC, 512], dt.bfloat16,
                          kind="ExternalInput")
    wq_d = nc.dram_tensor("wq", [128, NKC, CPC], dt.bfloat16, kind="ExternalInput")
    wk_d = nc.dram_tensor("wk", [128, NKC, CPC], dt.bfloat16, kind="ExternalInput")
    wv_d = nc.dram_tensor("wv", [128, NKC, CPC], dt.bfloat16, kind="ExternalInput")
    wp_d = nc.dram_tensor("wp", [128, NM, N_EMBD], dt.bfloat16,
                          kind="ExternalInput")
    bq_d = nc.dram_tensor("bq_r", [128, NM], dt.float32, kind="ExternalInput")
    bk_d = nc.dram_tensor("bk_r", [128, NM], dt.float32, kind="ExternalInput")
    bv_d = nc.dram_tensor("bv_bc", [128, CPC], dt.float32, kind="ExternalInput")
    mk_d = nc.dram_tensor("masks", [128, 4, 512], dt.bfloat16, kind="ExternalInput")

    kT_o = nc.dram_tensor("kT_out", [CPC, T], dt.bfloat16, kind="ExternalOutput")
    v_o = nc.dram_tensor("v_out", [T, CPC], dt.bfloat16, kind="ExternalOutput")
    yp_o = nc.dram_tensor("ypT_out", [N_EMBD, T], dt.bfloat16, kind="ExternalOutput")

    with tile.TileContext(nc) as tc:
        with tc.tile_pool(name="const", bufs=1) as cp, \
             tc.tile_pool(name="ex", bufs=6) as exp_pool, \
             tc.tile_pool(name="stage", bufs=4) as stage, \
             tc.tile_pool(name="small", bufs=4) as small:

            # ---- load inputs, ordered so compute can start early ----
            wq_sb = cp.tile([128, NKC, CPC], dt.bfloat16, tag="wq")
            nc.sync.dma_start(wq_sb, wq_d.ap())
            # x^T, bb-major: [p, bb, kc, 512]
            xT_sb = cp.tile([128, NB, NKC, 512], dt.bfloat16, tag="xT")
            for bb in range(NB):
                nc.sync.dma_start(xT_sb[:, bb], xT_d.ap()[bb])
            bq_sb = cp.tile([128, NM], dt.float32, tag="bq")
            nc.sync.dma_start(bq_sb, bq_d.ap())
            wk_sb = cp.tile([128, NKC, CPC], dt.bfloat16, tag="wk")
            nc.sync.dma_start(wk_sb, wk_d.ap())
            bk_sb = cp.tile([128, NM], dt.float32, tag="bk")
            nc.sync.dma_start(bk_sb, bk_d.ap())
            mk_sb = cp.tile([128, 4, 512], dt.bfloat16, tag="mk")
            nc.sync.dma_start(mk_sb, mk_d.ap())
            wv_sb = cp.tile([128, NKC, CPC], dt.bfloat16, tag="wv")
            nc.sync.dma_start(wv_sb, wv_d.ap())
            bv_sb = cp.tile([128, CPC], dt.float32, tag="bv")
            nc.sync.dma_start(bv_sb, bv_d.ap())
            wp_sb = cp.tile([128, NM, N_EMBD], dt.bfloat16, tag="wp")
            nc.sync.dma_start(wp_sb, wp_d.ap())

            qT_sb = cp.tile([128, NM, T], dt.bfloat16, tag="qT")
            kT_sb = cp.tile([128, NM, T], dt.bfloat16, tag="kT")
            # Vaug: [t%128, tt, head, 65] (col 64 = ones)
            v_sb = cp.tile([128, NTT, HPC, 65], dt.bfloat16, tag="v")
            yT_sb = cp.tile([128, NM, T], dt.bfloat16, tag="yT")

            nc.gpsimd.memset(v_sb[:, :, :, 64:65], 1.0)

            kT_o_r = kT_o.ap().rearrange("(m p) t -> p m t", p=128)
            v_o_r = v_o.ap().rearrange("(tt p) (h d) -> p tt h d", p=128, h=HPC)
            yp_r = yp_o.ap().rearrange("(mo p) t -> p mo t", p=128)

            # PSUM pools: sps + avps live throughout; qkps (QKV) is swapped
            # for pps (projection) once the last QK pair is emitted.
            sps_cm = tc.tile_pool(name="sps", bufs=2, space="PSUM")
            sps = sps_cm.__enter__()
            avps_cm = tc.tile_pool(name="avps", bufs=1, space="PSUM")
            avps = avps_cm.__enter__()
            qkps_cm = tc.tile_pool(name="qkps", bufs=2, space="PSUM")
            qkps = qkps_cm.__enter__()

            def qk_pair(m):
                for (w_sb, bias_sb, out_sb, nm) in ((wq_sb, bq_sb, qT_sb, "q"),
                                                    (wk_sb, bk_sb, kT_sb, "k")):
                    for bb in range(NB):
                        ps = qkps.tile([128, 512], dt.float32, tag="qk",
                                       name=f"{nm}_{m}_{bb}")
                        for kc in range(NKC):
                            nc.tensor.matmul(
                                ps,
                                lhsT=w_sb[:, kc, 128 * m:128 * m + 128],
                                rhs=xT_sb[:, bb, kc, :],
                                start=(kc == 0), stop=(kc == NKC - 1))
                        nc.vector.tensor_scalar(
                            out=out_sb[:, m, 512 * bb:512 * bb + 512],
                            in0=ps, scalar1=bias_sb[:, m:m + 1], scalar2=None,
                            op0=ALU.add)
                nc.sync.dma_start(kT_o_r[:, m], kT_sb[:, m])

            def v_tt(tt):
                ps = qkps.tile([128, 512], dt.float32, tag="qk", name=f"v_{tt}")
                for kc in range(NKC):
                    nc.tensor.matmul(
                        ps,
                        lhsT=xT_sb[:, tt // 4, kc, 128 * (tt % 4):128 * (tt % 4) + 128],
                        rhs=wv_sb[:, kc, :],
                        start=(kc == 0), stop=(kc == NKC - 1))
                nc.vector.tensor_tensor(
                    out=v_sb[:, tt, :, 0:64],
                    in0=ps.rearrange("p (h d) -> p h d", h=HPC),
                    in1=bv_sb.rearrange("p (h d) -> p h d", h=HPC),
                    op=ALU.add)
                nc.sync.dma_start(v_o_r[:, tt], v_sb[:, tt, :, 0:64])

            def attn_block(m, bb):
                njc = 4 * bb + 4  # causal s-chunk count for this t block
                ps_y = [avps.tile([65, 512], dt.float32, tag=f"av{h}",
                                  name=f"av_{h}_{m}_{bb}")
                        for h in range(2)]
                for j in range(njc):
                    # one PSUM tile holds BOTH heads' S^T for chunk j; the two
                    # K=64 row-tiled matmuls (tile_position rows 0/64) become
                    # ready together (single exp releases the slot), stay
                    # adjacent in the PE stream, and co-issue.
                    ps_s = sps.tile([128, 2, 512], dt.float32, tag="s",
                                    name=f"s_{m}_{bb}_{j}")
                    for half in range(2):
                        p0 = 64 * half
                        nc.tensor.matmul(
                            ps_s[:, half, :],
                            lhsT=kT_sb[p0:p0 + 64, m, 128 * j:128 * j + 128],
                            rhs=qT_sb[p0:p0 + 64, m, 512 * bb:512 * bb + 512],
                            start=True, stop=True)
                    ex = exp_pool.tile([128, 2, 512], dt.bfloat16,
                                       tag="ex", name=f"ex_{m}_{bb}_{j}")
                    nc.scalar.activation(ex, ps_s, AF.Exp, scale=0.125)
                    jpos = j - 4 * bb
                    if jpos >= 0:  # diagonal chunk: causal mask (both heads)
                        nc.vector.tensor_tensor(
                            ex, ex,
                            mk_sb[:, jpos:jpos + 1, :].to_broadcast(
                                [128, 2, 512]),
                            ALU.mult)
                    for half in range(2):
                        nc.tensor.matmul(
                            ps_y[half],
                            lhsT=v_sb[:, j, 2 * m + half, :],
                            rhs=ex[:, half, :],
                            start=(j == 0), stop=(j == njc - 1))
                for half in range(2):
                    row = small.tile([1, 512], dt.float32, tag="row",
                                     name=f"row_{m}_{bb}_{half}")
                    nc.vector.tensor_copy(row, ps_y[half][64:65, :])
                    bc = small.tile([64, 512], dt.float32, tag="bc",
                                    name=f"bc_{m}_{bb}_{half}")
                    nc.gpsimd.partition_broadcast(bc, row, channels=64)
                    rec = small.tile([64, 512], dt.float32, tag="rec",
                                     name=f"rec_{m}_{bb}_{half}")
                    nc.vector.reciprocal_approx_fast(out=rec, in_=bc)
                    nc.vector.tensor_tensor(
                        yT_sb[64 * half:64 * half + 64, m,
                              512 * bb:512 * bb + 512],
                        ps_y[half][0:64, :], rec, ALU.mult)

            # emission order = scheduling priority: start attention ASAP,
            # hide remaining QKV + projection under the ScalarE exp stream.
            qk_pair(0)
            for tt in range(4):
                v_tt(tt)
            attn_block(0, 0)
            qk_pair(1)
            for tt in range(4, 8):
                v_tt(tt)
            attn_block(0, 1)
            for tt in range(8, 12):
                v_tt(tt)
            attn_block(0, 2)
            for tt in range(12, 16):
                v_tt(tt)
            qk_pair(2)
            attn_block(0, 3)
            attn_block(1, 0)
            qk_pair(3)
            attn_block(1, 1)
            attn_block(1, 2)
            attn_block(1, 3)

            # QKV psum pool -> projection psum pool
            qkps_cm.__exit__(None, None, None)
            pps_cm = tc.tile_pool(name="pps", bufs=2, space="PSUM")
            pps = pps_cm.__enter__()

            def proj_bb(bb):
                for mo in range(N_EMBD // 128):
                    ps = pps.tile([128, 512], dt.float32, tag="pp",
                                  name=f"pp_{mo}_{bb}")
                    for kc in range(NM):
                        nc.tensor.matmul(
                            ps,
                            lhsT=wp_sb[:, kc, 128 * mo:128 * mo + 128],
                            rhs=yT_sb[:, kc, 512 * bb:512 * bb + 512],
                            start=(kc == 0), stop=(kc == NM - 1))
                    st = stage.tile([128, 512], dt.bfloat16, tag="st",
                                    name=f"st_{mo}_{bb}")
                    nc.vector.tensor_copy(st, ps)
                    nc.sync.dma_start(yp_r[:, mo, 512 * bb:512 * bb + 512], st)

            attn_block(2, 0)
            attn_block(2, 1)
            attn_block(2, 2)
            attn_block(2, 3)
            attn_block(3, 0)
            proj_bb(0)
            attn_block(3, 1)
            proj_bb(1)
            attn_block(3, 2)
            proj_bb(2)
            attn_block(3, 3)
            proj_bb(3)

            pps_cm.__exit__(None, None, None)
            avps_cm.__exit__(None, None, None)
            sps_cm.__exit__(None, None, None)

    _strip_self_waits(nc, mybir)
    nc.finalize()
    return nc


def _get_nc():
    if "nc" not in _BUILT:
        _BUILT["nc"] = _build_nc()
    return _BUILT["nc"]


def _make_masks():
    sp = np.arange(128)[:, None]
    tp = np.arange(512)[None, :]
    return np.stack([(tp >= 128 * jpos + sp) for jpos in range(4)],
                    axis=1).astype(BF16)  # [128, 4, 512]


def kernel(x, Wq, bq, Wk, bk, Wv, bv, Wp, bp):
    global LAST_RESULT
    from concourse.bass_utils import run_bass_kernel_spmd

    x = np.asarray(x, F32)
    Wq = np.asarray(Wq, F32); bq = np.asarray(bq, F32)
    Wk = np.asarray(Wk, F32); bk = np.asarray(bk, F32)
    Wv = np.asarray(Wv, F32); bv = np.asarray(bv, F32)
    Wp = np.asarray(Wp, F32); bp = np.asarray(bp, F32)

    nc = _get_nc()
    masks = _make_masks()
    xT = np.ascontiguousarray(x.transpose(0, 2, 1))  # [B, C, T]
    # [C, T] -> [NB, 128, NKC, 512]: (kc*128+p, bb*512+t') -> [bb, p, kc, t']
    xT_r = [np.ascontiguousarray(
        xT[b].reshape(NKC, 128, NB, 512).transpose(2, 1, 0, 3)).astype(BF16)
        for b in range(B)]

    def warr(w):  # [1024, 512] -> [128, NKC, 512]
        return np.ascontiguousarray(
            w.reshape(NKC, 128, CPC).transpose(1, 0, 2)).astype(BF16)

    in_maps = []
    for c in range(N_CORES):
        b, g = divmod(c, 2)
        sl = slice(CPC * g, CPC * g + CPC)
        in_maps.append({
            "xT": xT_r[b],
            "wq": warr(Wq[:, sl]),
            "wk": warr(Wk[:, sl]),
            "wv": warr(Wv[:, sl]),
            "wp": np.ascontiguousarray(
                Wp[sl, :].reshape(NM, 128, N_EMBD).transpose(1, 0, 2)).astype(BF16),
            "bq_r": np.ascontiguousarray(bq[sl].reshape(NM, 128).T),
            "bk_r": np.ascontiguousarray(bk[sl].reshape(NM, 128).T),
            "bv_bc": np.ascontiguousarray(
                np.broadcast_to(bv[sl], (128, CPC))).astype(F32),
            "masks": masks,
        })

    res = run_bass_kernel_spmd(nc, in_maps, core_ids=list(range(N_CORES)))
    LAST_RESULT = res

    y = np.empty((B, T, N_EMBD), F32)
    k = np.empty((B, N_HEAD, T, HEAD_DIM), F32)
    v = np.empty((B, N_HEAD, T, HEAD_DIM), F32)
    for c in range(N_CORES):
        b, g = divmod(c, 2)
        out = res.results[c]
        kT = out["kT_out"].astype(F32)           # [512, T]
        vn = out["v_out"].astype(F32)            # [T, 512]
        for lh in range(HPC):
            h = HPC * g + lh
            k[b, h] = kT[64 * lh:64 * lh + 64, :].T
            v[b, h] = vn[:, 64 * lh:64 * lh + 64]
    for b in range(B):
        ypT = (res.results[2 * b]["ypT_out"].astype(F32)
               + res.results[2 * b + 1]["ypT_out"].astype(F32))  # [C, T]
        y[b] = ypT.T + bp[None, :]

    present = np.stack([k, v])  # [2, B, H, T, D]
    return y, present


# revision 20
# speedup vs baseline: 1.1551x; 1.1305x over previous
"""Causal self-attention (B=4, T=2048, C=1024, H=16, D=64) on 8 TRN2 NeuronCores.

Sharding: core c -> (batch b = c//2, head-group g = c%2 covering heads
8g..8g+8). Data-parallel over B, tensor-parallel over heads. The output
projection is computed per-core over its 512 channels; the two partial
products per batch are summed on the host (the "all-reduce"), where the
projection bias is also added.

Per-core kernel (single SPMD program, per-core data):
  - qT = (x Wq + bq)^T and kT likewise, laid out [c'=512, T] (head-major on
    partitions: chunk m holds heads 2m, 2m+1 at partition offsets 0/64); v in
    natural layout [T, c'] with an appended ones column per head (Vaug, M=65)
    for the softmax denominator.
  - attention per head pair m, per 512-wide t-block: S^T[s,t] tiles via K=64
    matmuls packed two-heads-per-PE-array (tile_position derives from
    base_partition 0/64); exp on ScalarE (scale=1/8 folded) straight from PSUM
    into bf16 SBUF; causal masking by multiplying constant triangular masks on
    the diagonal s-chunks; AV matmuls with lhsT=Vaug -> unnormalized y^T (rows
    0:64) and sumexp (row 64) in one PSUM accumulation; normalize with
    partition_broadcast + fast reciprocal.
  - y_out^T = Wp_g^T @ yT accumulated over the core's 4 channel chunks,
    streamed to DRAM as bf16.

QKV compute, attention, and the projection are interleaved so the ScalarE
(exp) stream starts early and PE work hides under it. After Tile scheduling,
redundant same-engine semaphore waits are stripped so cross-engine waits ride
the instructions themselves instead of spawning EVENT_SEMAPHORE ops on the
busy Scalar queue.

No max-subtraction in softmax: scores are O(1) here (exp is safe in fp32),
and exp(S)/sum(exp(S)) is mathematically identical to jax.nn.softmax.
"""

from functools import partial

import numpy as np
import ml_dtypes

BF16 = ml_dtypes.bfloat16
F32 = np.float32

N_EMBD = 1024
N_HEAD = 16
HEAD_DIM = 64
B = 4
T = 2048
N_CORES = 8
HPC = 8          # heads per core
CPC = HPC * HEAD_DIM  # channels per core = 512
NKC = N_EMBD // 128   # contraction chunks over full embed = 8
NM = CPC // 128       # head-pair chunks per core = 4
NB = T // 512         # 512-wide t blocks = 4
NTT = T // 128        # 128-wide t tiles = 16

_BUILT = {}
LAST_RESULT = None  # BassKernelResults of the most recent run (for test harness)


def _strip_self_waits(nc, mybir):
    """Remove same-engine semaphore waits (vacuous on an in-order queue for
    equal-shape streaming ops) so the single HW wait slot is free for the
    real cross-engine dependency."""
    pfx = {
        mybir.EngineType.Activation: "Activation_",
        mybir.EngineType.DVE: "DVE_",
        mybir.EngineType.PE: "PE_",
        mybir.EngineType.Pool: "Pool_",
        mybir.EngineType.SP: "SP_",
    }
    n = 0
    for blk in nc.main_func.blocks:
        for ins in blk.instructions:
            si = ins.sync_info
            if si is None or not si.on_wait:
                continue
            p = pfx.get(ins.engine)
            if not p:
                continue
            kept = [w for w in si.on_wait
                    if not (w.ant_name and w.ant_name.startswith(p)
                            and w.wait_mode == "sem-ge-imm")]
            if len(kept) != len(si.on_wait):
                n += len(si.on_wait) - len(kept)
                si.on_wait = kept
    return n


def _build_nc():
    import concourse.bass as bass
    import concourse.mybir as mybir
    import concourse.tile as tile
    from concourse import bacc

    dt = mybir.dt
    AF = mybir.ActivationFunctionType
    ALU = mybir.AluOpType

    nc = bacc.Bacc(trn_type="TRN2", name="csa")

    # ---- DRAM I/O (host pre-arranges for contiguous per-partition DMA) ----
    xT_d = nc.dram_tensor("xT", [NB, 128, NKC, 512], dt.bfloat16,
                          kind="ExternalInput")
    wq_d = nc.dram_tensor("wq", [128, NKC, CPC], dt.bfloat16, kind="ExternalInput")
    wk_d = nc.dram_tensor("wk", [128, NKC, CPC], dt.bfloat16, kind="ExternalInput")
    wv_d = nc.dram_tensor("wv", [128, NKC, CPC], dt.bfloat16, kind="ExternalInput")
    wp_d = nc.dram_tensor("wp", [128, NM, N_EMBD], dt.bfloat16,
                          kind="ExternalInput")
    bq_d = nc.dram_tensor("bq_r", [128, NM], dt.float32, kind="ExternalInput")
    bk_d = nc.dram_tensor("bk_r", [128, NM], dt.float32, kind="ExternalInput")
    bv_d = nc.dram_tensor("bv_bc", [128, CPC], dt.float32, kind="ExternalInput")
    mk_d = nc.dram_tensor("masks", [128, 4, 512], dt.bfloat16, kind="ExternalInput")

    kT_o = nc.dram_tensor("kT_out", [CPC, T], dt.bfloat16, kind="ExternalOutput")
    v_o = nc.dram_tensor("v_out", [T, CPC], dt.bfloat16, kind="ExternalOutput")
    yp_o = nc.dram_tensor("ypT_out", [N_EMBD, T], dt.bfloat16, kind="ExternalOutput")

    with tile.TileContext(nc) as tc:
        with tc.tile_pool(name="const", bufs=1) as cp, \
             tc.tile_pool(name="ex", bufs=6) as exp_pool, \
             tc.tile_pool(name="stage", bufs=4) as stage, \
             tc.tile_pool(name="small", bufs=4) as small:

            # ---- load inputs, ordered so compute can start early ----
            wq_sb = cp.tile([128, NKC, CPC], dt.bfloat16, tag="wq")
            nc.sync.dma_start(wq_sb, wq_d.ap())
            # x^T, bb-major: [p, bb, kc, 512]
            xT_sb = cp.tile([128, NB, NKC, 512], dt.bfloat16, tag="xT")
            for bb in range(NB):
                nc.sync.dma_start(xT_sb[:, bb], xT_d.ap()[bb])
            bq_sb = cp.tile([128, NM], dt.float32, tag="bq")
            nc.sync.dma_start(bq_sb, bq_d.ap())
            wk_sb = cp.tile([128, NKC, CPC], dt.bfloat16, tag="wk")
            nc.sync.dma_start(wk_sb, wk_d.ap())
            bk_sb = cp.tile([128, NM], dt.float32, tag="bk")
            nc.sync.dma_start(bk_sb, bk_d.ap())
            mk_sb = cp.tile([128, 4, 512], dt.bfloat16, tag="mk")
            nc.sync.dma_start(mk_sb, mk_d.ap())
            wv_sb = cp.tile([128, NKC, CPC], dt.bfloat16, tag="wv")
            nc.sync.dma_start(wv_sb, wv_d.ap())
            bv_sb = cp.tile([128, CPC], dt.float32, tag="bv")
            nc.sync.dma_start(bv_sb, bv_d.ap())
            wp_sb = cp.tile([128, NM, N_EMBD], dt.bfloat16, tag="wp")
            nc.sync.dma_start(wp_sb, wp_d.ap())

            qT_sb = cp.tile([128, NM, T], dt.bfloat16, tag="qT")
            kT_sb = cp.tile([128, NM, T], dt.bfloat16, tag="kT")
            # Vaug: [t%128, tt, head, 65] (col 64 = ones)
            v_sb = cp.tile([128, NTT, HPC, 65], dt.bfloat16, tag="v")
            yT_sb = cp.tile([128, NM, T], dt.bfloat16, tag="yT")

            nc.gpsimd.memset(v_sb[:, :, :, 64:65], 1.0)

            kT_o_r = kT_o.ap().rearrange("(m p) t -> p m t", p=128)
            v_o_r = v_o.ap().rearrange("(tt p) (h d) -> p tt h d", p=128, h=HPC)
            yp_r = yp_o.ap().rearrange("(mo p) t -> p mo t", p=128)

            # PSUM pools: sps + avps live throughout; qkps (QKV) is swapped
            # for pps (projection) once the last QK pair is emitted.
            sps_cm = tc.tile_pool(name="sps", bufs=2, space="PSUM")
            sps = sps_cm.__enter__()
            avps_cm = tc.tile_pool(name="avps", bufs=1, space="PSUM")
            avps = avps_cm.__enter__()
            qkps_cm = tc.tile_pool(name="qkps", bufs=2, space="PSUM")
            qkps = qkps_cm.__enter__()

            def qk_chain(which, m, bb):
                w_sb, bias_sb, out_sb = ((wq_sb, bq_sb, qT_sb) if which == "q"
                                         else (wk_sb, bk_sb, kT_sb))
                ps = qkps.tile([128, 512], dt.float32, tag="qk",
                               name=f"{which}_{m}_{bb}")
                for kc in range(NKC):
                    nc.tensor.matmul(
                        ps,
                        lhsT=w_sb[:, kc, 128 * m:128 * m + 128],
                        rhs=xT_sb[:, bb, kc, :],
                        start=(kc == 0), stop=(kc == NKC - 1))
                nc.vector.tensor_scalar(
                    out=out_sb[:, m, 512 * bb:512 * bb + 512],
                    in0=ps, scalar1=bias_sb[:, m:m + 1], scalar2=None,
                    op0=ALU.add)
                if which == "k":
                    nc.sync.dma_start(kT_o_r[:, m, 512 * bb:512 * bb + 512],
                                      kT_sb[:, m, 512 * bb:512 * bb + 512])

            def v_chain(tt):
                ps = qkps.tile([128, 512], dt.float32, tag="qk", name=f"v_{tt}")
                for kc in range(NKC):
                    nc.tensor.matmul(
                        ps,
                        lhsT=xT_sb[:, tt // 4, kc, 128 * (tt % 4):128 * (tt % 4) + 128],
                        rhs=wv_sb[:, kc, :],
                        start=(kc == 0), stop=(kc == NKC - 1))
                nc.vector.tensor_tensor(
                    out=v_sb[:, tt, :, 0:64],
                    in0=ps.rearrange("p (h d) -> p h d", h=HPC),
                    in1=bv_sb.rearrange("p (h d) -> p h d", h=HPC),
                    op=ALU.add)
                nc.sync.dma_start(v_o_r[:, tt], v_sb[:, tt, :, 0:64])

            def attn_block(m, bb, fillers=()):
                njc = 4 * bb + 4  # causal s-chunk count for this t block
                ps_y = [avps.tile([65, 512], dt.float32, tag=f"av{h}",
                                  name=f"av_{h}_{m}_{bb}")
                        for h in range(2)]
                fit = iter(fillers)
                for j in range(njc):
                    f = next(fit, None)
                    if f is not None:
                        f()
                    jpos = j - 4 * bb
                    toff = 128 * jpos if jpos > 0 else 0
                    W = 512 - toff  # causally-valid width of this chunk
                    # one PSUM tile holds BOTH heads' S^T for chunk j; the two
                    # K=64 row-tiled matmuls (tile_position rows 0/64) become
                    # ready together (single exp releases the slot), stay
                    # adjacent in the PE stream, and co-issue.
                    ps_s = sps.tile([128, 2, 512], dt.float32, tag="s",
                                    name=f"s_{m}_{bb}_{j}")
                    for half in range(2):
                        p0 = 64 * half
                        nc.tensor.matmul(
                            ps_s[:, half, 0:W],
                            lhsT=kT_sb[p0:p0 + 64, m, 128 * j:128 * j + 128],
                            rhs=qT_sb[p0:p0 + 64, m,
                                      512 * bb + toff:512 * bb + 512],
                            start=True, stop=True)
                    ex = exp_pool.tile([128, 2, 512], dt.bfloat16,
                                       tag="ex", name=f"ex_{m}_{bb}_{j}")
                    nc.scalar.activation(ex[:, :, 0:W], ps_s[:, :, 0:W],
                                         AF.Exp, scale=0.125)
                    if jpos >= 0:  # diagonal chunk: causal mask (both heads)
                        nc.vector.tensor_tensor(
                            ex[:, :, 0:W], ex[:, :, 0:W],
                            mk_sb[:, jpos:jpos + 1, toff:512].to_broadcast(
                                [128, 2, W]),
                            ALU.mult)
                    for half in range(2):
                        nc.tensor.matmul(
                            ps_y[half][:, toff:512],
                            lhsT=v_sb[:, j, 2 * m + half, :],
                            rhs=ex[:, half, 0:W],
                            start=(j == 0), stop=(j == njc - 1),
                            skip_group_check=True)
                for f in fit:  # any leftover fillers
                    f()
                for half in range(2):
                    # stage PSUM reads first so the AV slot frees quickly
                    row = small.tile([1, 512], dt.float32, tag="row",
                                     name=f"row_{m}_{bb}_{half}")
                    nc.vector.tensor_copy(row, ps_y[half][64:65, :])
                    stg = small.tile([64, 512], dt.float32, tag="stg",
                                     name=f"stg_{m}_{bb}_{half}")
                    nc.vector.tensor_copy(stg, ps_y[half][0:64, :])
                    bc = small.tile([64, 512], dt.float32, tag="bc",
                                    name=f"bc_{m}_{bb}_{half}")
                    nc.gpsimd.partition_broadcast(bc, row, channels=64)
                    rec = small.tile([64, 512], dt.float32, tag="rec",
                                     name=f"rec_{m}_{bb}_{half}")
                    nc.vector.reciprocal_approx_fast(out=rec, in_=bc)
                    nc.vector.tensor_tensor(
                        yT_sb[64 * half:64 * half + 64, m,
                              512 * bb:512 * bb + 512],
                        stg, rec, ALU.mult)

            def proj_chain(mo, bb):
                ps = pps.tile([128, 512], dt.float32, tag="pp",
                              name=f"pp_{mo}_{bb}")
                for kc in range(NM):
                    nc.tensor.matmul(
                        ps,
                        lhsT=wp_sb[:, kc, 128 * mo:128 * mo + 128],
                        rhs=yT_sb[:, kc, 512 * bb:512 * bb + 512],
                        start=(kc == 0), stop=(kc == NM - 1))
                st = stage.tile([128, 512], dt.bfloat16, tag="st",
                                name=f"st_{mo}_{bb}")
                nc.vector.tensor_copy(st, ps)
                nc.sync.dma_start(yp_r[:, mo, 512 * bb:512 * bb + 512], st)

            # Filler queue: QKV chains drip-fed into the attention j-loops so
            # PE never monopolizes long stretches while ScalarE starves.
            work = []
            for bb in range(1, NB):
                work += [partial(qk_chain, "q", 0, bb),
                         partial(qk_chain, "k", 0, bb)]
                work += [partial(v_chain, tt) for tt in range(4 * bb, 4 * bb + 4)]
            for m in range(1, NM):
                for bb in range(NB):
                    work += [partial(qk_chain, "q", m, bb),
                             partial(qk_chain, "k", m, bb)]
            wq_i = iter(work)

            def take(n):
                out = []
                for _ in range(n):
                    f = next(wq_i, None)
                    if f is None:
                        break
                    out.append(f)
                return out

            # prelude: minimum needed for attn(0,0)
            qk_chain("q", 0, 0)
            qk_chain("k", 0, 0)
            for tt in range(4):
                v_chain(tt)

            attn_block(0, 0, take(4))
            attn_block(0, 1, take(6))
            attn_block(0, 2, take(8))
            attn_block(0, 3, take(10))
            attn_block(1, 0, take(2))
            attn_block(1, 1, take(4))
            attn_block(1, 2, take(6))
            attn_block(1, 3, take(8))
            attn_block(2, 0, take(2))
            attn_block(2, 1, take(4))
            for f in wq_i:  # any remaining QKV chains
                f()

            # QKV psum pool -> projection psum pool (all QKV chains emitted)
            qkps_cm.__exit__(None, None, None)
            pps_cm = tc.tile_pool(name="pps", bufs=2, space="PSUM")
            pps = pps_cm.__enter__()

            attn_block(2, 2)
            attn_block(2, 3)
            attn_block(3, 0)
            attn_block(3, 1, [partial(proj_chain, mo, 0)
                              for mo in range(N_EMBD // 128)])
            attn_block(3, 2, [partial(proj_chain, mo, 1)
                              for mo in range(N_EMBD // 128)])
            attn_block(3, 3, [partial(proj_chain, mo, 2)
                              for mo in range(N_EMBD // 128)])
            for mo in range(N_EMBD // 128):
                proj_chain(mo, 3)

            pps_cm.__exit__(None, None, None)
            avps_cm.__exit__(None, None, None)
            sps_cm.__exit__(None, None, None)

    _strip_self_waits(nc, mybir)
    nc.finalize()
    return nc


def _get_nc():
    if "nc" not in _BUILT:
        _BUILT["nc"] = _build_nc()
    return _BUILT["nc"]


def _make_masks():
    sp = np.arange(128)[:, None]
    tp = np.arange(512)[None, :]
    return np.stack([(tp >= 128 * jpos + sp) for jpos in range(4)],
                    axis=1).astype(BF16)  # [128, 4, 512]


def kernel(x, Wq, bq, Wk, bk, Wv, bv, Wp, bp):
    global LAST_RESULT
    from concourse.bass_utils import run_bass_kernel_spmd

    x = np.asarray(x, F32)
    Wq = np.asarray(Wq, F32); bq = np.asarray(bq, F32)
    Wk = np.asarray(Wk, F32); bk = np.asarray(bk, F32)
    Wv = np.asarray(Wv, F32); bv = np.asarray(bv, F32)
    Wp = np.asarray(Wp, F32); bp = np.asarray(bp, F32)

    nc = _get_nc()
    masks = _make_masks()
    xT = np.ascontiguousarray(x.transpose(0, 2, 1))  # [B, C, T]
    # [C, T] -> [NB, 128, NKC, 512]: (kc*128+p, bb*512+t') -> [bb, p, kc, t']
    xT_r = [np.ascontiguousarray(
        xT[b].reshape(NKC, 128, NB, 512).transpose(2, 1, 0, 3)).astype(BF16)
        for b in range(B)]

    def warr(w):  # [1024, 512] -> [128, NKC, 512]
        return np.ascontiguousarray(
            w.reshape(NKC, 128, CPC).transpose(1, 0, 2)).astype(BF16)

    in_maps = []
    for c in range(N_CORES):
        b, g = divmod(c, 2)
        sl = slice(CPC * g, CPC * g + CPC)
        in_maps.append({
            "xT": xT_r[b],
            "wq": warr(Wq[:, sl]),
            "wk": warr(Wk[:, sl]),
            "wv": warr(Wv[:, sl]),
            "wp": np.ascontiguousarray(
                Wp[sl, :].reshape(NM, 128, N_EMBD).transpose(1, 0, 2)).astype(BF16),
            "bq_r": np.ascontiguousarray(bq[sl].reshape(NM, 128).T),
            "bk_r": np.ascontiguousarray(bk[sl].reshape(NM, 128).T),
            "bv_bc": np.ascontiguousarray(
                np.broadcast_to(bv[sl], (128, CPC))).astype(F32),
            "masks": masks,
        })

    res = run_bass_kernel_spmd(nc, in_maps, core_ids=list(range(N_CORES)))
    LAST_RESULT = res

    y = np.empty((B, T, N_EMBD), F32)
    k = np.empty((B, N_HEAD, T, HEAD_DIM), F32)
    v = np.empty((B, N_HEAD, T, HEAD_DIM), F32)
    for c in range(N_CORES):
        b, g = divmod(c, 2)
        out = res.results[c]
        kT = out["kT_out"].astype(F32)           # [512, T]
        vn = out["v_out"].astype(F32)            # [T, 512]
        for lh in range(HPC):
            h = HPC * g + lh
            k[b, h] = kT[64 * lh:64 * lh + 64, :].T
            v[b, h] = vn[:, 64 * lh:64 * lh + 64]
    for b in range(B):
        ypT = (res.results[2 * b]["ypT_out"].astype(F32)
               + res.results[2 * b + 1]["ypT_out"].astype(F32))  # [C, T]
        y[b] = ypT.T + bp[None, :]

    present = np.stack([k, v])  # [2, B, H, T, D]
    return y, present


# revision 26
# speedup vs baseline: 1.1994x; 1.0384x over previous
"""Causal self-attention (B=4, T=2048, C=1024, H=16, D=64) on 8 TRN2 NeuronCores.

Sharding: core c -> (batch b = c//2, head-group g = c%2 covering heads
8g..8g+8). Data-parallel over B, tensor-parallel over heads. The output
projection is computed per-core over its 512 channels; the two partial
products per batch are summed on the host (the "all-reduce"), where the
projection bias is also added.

Per-core kernel (single SPMD program, per-core data):
  - qT = (x Wq + bq)^T and kT likewise, laid out [c'=512, T] (head-major on
    partitions: chunk m holds heads 2m, 2m+1 at partition offsets 0/64); v in
    natural layout [T, c'] with an appended ones column per head (Vaug, M=65)
    for the softmax denominator.
  - attention per head pair m, per 512-wide t-block: S^T[s,t] tiles via K=64
    matmuls packed two-heads-per-PE-array (tile_position derives from
    base_partition 0/64); exp on ScalarE (scale=1/8 folded) straight from PSUM
    into bf16 SBUF; causal masking by multiplying constant triangular masks on
    the diagonal s-chunks; AV matmuls with lhsT=Vaug -> unnormalized y^T (rows
    0:64) and sumexp (row 64) in one PSUM accumulation; normalize with
    partition_broadcast + fast reciprocal.
  - y_out^T = Wp_g^T @ yT accumulated over the core's 4 channel chunks,
    streamed to DRAM as bf16.

QKV compute, attention, and the projection are interleaved so the ScalarE
(exp) stream starts early and PE work hides under it. After Tile scheduling,
redundant same-engine semaphore waits are stripped so cross-engine waits ride
the instructions themselves instead of spawning EVENT_SEMAPHORE ops on the
busy Scalar queue.

No max-subtraction in softmax: scores are O(1) here (exp is safe in fp32),
and exp(S)/sum(exp(S)) is mathematically identical to jax.nn.softmax.
"""

from functools import partial

import numpy as np
import ml_dtypes

BF16 = ml_dtypes.bfloat16
F32 = np.float32

N_EMBD = 1024
N_HEAD = 16
HEAD_DIM = 64
B = 4
T = 2048
N_CORES = 8
HPC = 8          # heads per core
CPC = HPC * HEAD_DIM  # channels per core = 512
NKC = N_EMBD // 128   # contraction chunks over full embed = 8
NM = CPC // 128       # head-pair chunks per core = 4
NB = T // 512         # 512-wide t blocks = 4
NTT = T // 128        # 128-wide t tiles = 16

_BUILT = {}
LAST_RESULT = None  # BassKernelResults of the most recent run (for test harness)


def _strip_self_waits(nc, mybir):
    """Remove same-engine semaphore waits (vacuous on an in-order queue for
    equal-shape streaming ops) so the single HW wait slot is free for the
    real cross-engine dependency."""
    pfx = {
        mybir.EngineType.Activation: "Activation_",
        mybir.EngineType.DVE: "DVE_",
        mybir.EngineType.PE: "PE_",
        mybir.EngineType.Pool: "Pool_",
        mybir.EngineType.SP: "SP_",
    }
    n = 0
    for blk in nc.main_func.blocks:
        for ins in blk.instructions:
            si = ins.sync_info
            if si is None or not si.on_wait:
                continue
            p = pfx.get(ins.engine)
            if not p:
                continue
            kept = [w for w in si.on_wait
                    if not (w.ant_name and w.ant_name.startswith(p)
                            and w.wait_mode == "sem-ge-imm")]
            if len(kept) != len(si.on_wait):
                n += len(si.on_wait) - len(kept)
                si.on_wait = kept
    return n


def _build_nc():
    import concourse.bass as bass
    import concourse.mybir as mybir
    import concourse.tile as tile
    from concourse import bacc

    dt = mybir.dt
    AF = mybir.ActivationFunctionType
    ALU = mybir.AluOpType

    nc = bacc.Bacc(trn_type="TRN2", name="csa")

    # ---- DRAM I/O (host pre-arranges for contiguous per-partition DMA) ----
    xT_d = nc.dram_tensor("xT", [NB, 128, NKC, 512], dt.bfloat16,
                          kind="ExternalInput")
    wq_d = nc.dram_tensor("wq", [NM, 128, NKC, 128], dt.bfloat16,
                          kind="ExternalInput")
    wk_d = nc.dram_tensor("wk", [NM, 128, NKC, 128], dt.bfloat16,
                          kind="ExternalInput")
    wv_d = nc.dram_tensor("wv", [128, NKC, CPC], dt.bfloat16, kind="ExternalInput")
    wp_d = nc.dram_tensor("wp", [128, NM, N_EMBD], dt.bfloat16,
                          kind="ExternalInput")
    bq_d = nc.dram_tensor("bq_r", [128, NM], dt.float32, kind="ExternalInput")
    bk_d = nc.dram_tensor("bk_r", [128, NM], dt.float32, kind="ExternalInput")
    bv_d = nc.dram_tensor("bv_bc", [128, CPC], dt.float32, kind="ExternalInput")
    mk_d = nc.dram_tensor("masks", [128, 4, 512], dt.bfloat16, kind="ExternalInput")

    kT_o = nc.dram_tensor("kT_out", [CPC, T], dt.bfloat16, kind="ExternalOutput")
    v_o = nc.dram_tensor("v_out", [T, CPC], dt.bfloat16, kind="ExternalOutput")
    yp_o = nc.dram_tensor("ypT_out", [N_EMBD, T], dt.bfloat16, kind="ExternalOutput")

    with tile.TileContext(nc) as tc:
        with tc.tile_pool(name="const", bufs=1) as cp, \
             tc.tile_pool(name="ex", bufs=6) as exp_pool, \
             tc.tile_pool(name="stage", bufs=4) as stage, \
             tc.tile_pool(name="small", bufs=4) as small:

            # ---- load inputs, ordered so compute can start early ----
            # m-major weight tiles so the first chain's weights arrive fast
            wq_sb = cp.tile([128, NM, NKC, 128], dt.bfloat16, tag="wq")
            nc.sync.dma_start(wq_sb[:, 0], wq_d.ap()[0])
            # x^T, bb-major: [p, bb, kc, 512]
            xT_sb = cp.tile([128, NB, NKC, 512], dt.bfloat16, tag="xT")
            nc.sync.dma_start(xT_sb[:, 0], xT_d.ap()[0])
            wk_sb = cp.tile([128, NM, NKC, 128], dt.bfloat16, tag="wk")
            nc.sync.dma_start(wk_sb[:, 0], wk_d.ap()[0])
            bq_sb = cp.tile([128, NM], dt.float32, tag="bq")
            nc.sync.dma_start(bq_sb, bq_d.ap())
            bk_sb = cp.tile([128, NM], dt.float32, tag="bk")
            nc.sync.dma_start(bk_sb, bk_d.ap())
            for bb in range(1, NB):
                nc.sync.dma_start(xT_sb[:, bb], xT_d.ap()[bb])
            for m in range(1, NM):
                nc.sync.dma_start(wq_sb[:, m], wq_d.ap()[m])
                nc.sync.dma_start(wk_sb[:, m], wk_d.ap()[m])
            mk_sb = cp.tile([128, 4, 512], dt.bfloat16, tag="mk")
            nc.sync.dma_start(mk_sb, mk_d.ap())
            wv_sb = cp.tile([128, NKC, CPC], dt.bfloat16, tag="wv")
            nc.sync.dma_start(wv_sb, wv_d.ap())
            bv_sb = cp.tile([128, CPC], dt.float32, tag="bv")
            nc.sync.dma_start(bv_sb, bv_d.ap())
            wp_sb = cp.tile([128, NM, N_EMBD], dt.bfloat16, tag="wp")
            nc.sync.dma_start(wp_sb, wp_d.ap())

            qT_sb = cp.tile([128, NM, T], dt.bfloat16, tag="qT")
            kT_sb = cp.tile([128, NM, T], dt.bfloat16, tag="kT")
            # Vaug: [t%128, tt, head, 65] (col 64 = ones)
            v_sb = cp.tile([128, NTT, HPC, 65], dt.bfloat16, tag="v")
            yT_sb = cp.tile([128, NM, T], dt.bfloat16, tag="yT")

            nc.gpsimd.memset(v_sb[:, :, :, 64:65], 1.0)

            kT_o_r = kT_o.ap().rearrange("(m p) t -> p m t", p=128)
            v_o_r = v_o.ap().rearrange("(tt p) (h d) -> p tt h d", p=128, h=HPC)
            yp_r = yp_o.ap().rearrange("(mo p) t -> p mo t", p=128)

            # PSUM pools: sps + avps live throughout; qkps (QKV) is swapped
            # for pps (projection) once the last QK pair is emitted.
            sps_cm = tc.tile_pool(name="sps", bufs=2, space="PSUM")
            sps = sps_cm.__enter__()
            avps_cm = tc.tile_pool(name="avps", bufs=1, space="PSUM")
            avps = avps_cm.__enter__()
            qkps_cm = tc.tile_pool(name="qkps", bufs=2, space="PSUM")
            qkps = qkps_cm.__enter__()

            def qk_chain(which, m, bb):
                w_sb, bias_sb, out_sb = ((wq_sb, bq_sb, qT_sb) if which == "q"
                                         else (wk_sb, bk_sb, kT_sb))
                ps = qkps.tile([128, 512], dt.float32, tag="qk",
                               name=f"{which}_{m}_{bb}")
                for kc in range(NKC):
                    nc.tensor.matmul(
                        ps,
                        lhsT=w_sb[:, m, kc, :],
                        rhs=xT_sb[:, bb, kc, :],
                        start=(kc == 0), stop=(kc == NKC - 1))
                nc.vector.tensor_scalar(
                    out=out_sb[:, m, 512 * bb:512 * bb + 512],
                    in0=ps, scalar1=bias_sb[:, m:m + 1], scalar2=None,
                    op0=ALU.add)
                if which == "k":
                    nc.sync.dma_start(kT_o_r[:, m, 512 * bb:512 * bb + 512],
                                      kT_sb[:, m, 512 * bb:512 * bb + 512])

            def v_chain(tt):
                ps = qkps.tile([128, 512], dt.float32, tag="qk", name=f"v_{tt}")
                for kc in range(NKC):
                    nc.tensor.matmul(
                        ps,
                        lhsT=xT_sb[:, tt // 4, kc, 128 * (tt % 4):128 * (tt % 4) + 128],
                        rhs=wv_sb[:, kc, :],
                        start=(kc == 0), stop=(kc == NKC - 1))
                nc.vector.tensor_tensor(
                    out=v_sb[:, tt, :, 0:64],
                    in0=ps.rearrange("p (h d) -> p h d", h=HPC),
                    in1=bv_sb.rearrange("p (h d) -> p h d", h=HPC),
                    op=ALU.add)
                nc.sync.dma_start(v_o_r[:, tt], v_sb[:, tt, :, 0:64])

            def attn_block(m, bb, fillers=()):
                njc = 4 * bb + 4  # causal s-chunk count for this t block
                ps_y = [avps.tile([65, 512], dt.float32, tag=f"av{h}",
                                  name=f"av_{h}_{m}_{bb}")
                        for h in range(2)]
                fit = iter(fillers)
                for j in range(njc):
                    f = next(fit, None)
                    if f is not None:
                        f()
                    jpos = j - 4 * bb
                    toff = 128 * jpos if jpos > 0 else 0
                    W = 512 - toff  # causally-valid width of this chunk
                    # one PSUM tile holds BOTH heads' S^T for chunk j; the two
                    # K=64 row-tiled matmuls (tile_position rows 0/64) become
                    # ready together (single exp releases the slot), stay
                    # adjacent in the PE stream, and co-issue.
                    ps_s = sps.tile([128, 2, 512], dt.float32, tag="s",
                                    name=f"s_{m}_{bb}_{j}")
                    for half in range(2):
                        p0 = 64 * half
                        nc.tensor.matmul(
                            ps_s[:, half, 0:W],
                            lhsT=kT_sb[p0:p0 + 64, m, 128 * j:128 * j + 128],
                            rhs=qT_sb[p0:p0 + 64, m,
                                      512 * bb + toff:512 * bb + 512],
                            start=True, stop=True)
                    ex = exp_pool.tile([128, 2, 512], dt.bfloat16,
                                       tag="ex", name=f"ex_{m}_{bb}_{j}")
                    nc.scalar.activation(ex[:, :, 0:W], ps_s[:, :, 0:W],
                                         AF.Exp, scale=0.125)
                    if jpos >= 0:  # diagonal chunk: causal mask (both heads)
                        nc.vector.tensor_tensor(
                            ex[:, :, 0:W], ex[:, :, 0:W],
                            mk_sb[:, jpos:jpos + 1, toff:512].to_broadcast(
                                [128, 2, W]),
                            ALU.mult)
                    for half in range(2):
                        nc.tensor.matmul(
                            ps_y[half][:, toff:512],
                            lhsT=v_sb[:, j, 2 * m + half, :],
                            rhs=ex[:, half, 0:W],
                            start=(j == 0), stop=(j == njc - 1),
                            skip_group_check=True)
                for f in fit:  # any leftover fillers
                    f()
                for half in range(2):
                    # stage PSUM reads first so the AV slot frees quickly
                    row = small.tile([1, 512], dt.float32, tag="row",
                                     name=f"row_{m}_{bb}_{half}")
                    nc.vector.tensor_copy(row, ps_y[half][64:65, :])
                    stg = small.tile([64, 512], dt.float32, tag="stg",
                                     name=f"stg_{m}_{bb}_{half}")
                    nc.vector.tensor_copy(stg, ps_y[half][0:64, :])
                    bc = small.tile([64, 512], dt.float32, tag="bc",
                                    name=f"bc_{m}_{bb}_{half}")
                    nc.gpsimd.partition_broadcast(bc, row, channels=64)
                    rec = small.tile([64, 512], dt.float32, tag="rec",
                                     name=f"rec_{m}_{bb}_{half}")
                    nc.vector.reciprocal_approx_fast(out=rec, in_=bc)
                    nc.vector.tensor_tensor(
                        yT_sb[64 * half:64 * half + 64, m,
                              512 * bb:512 * bb + 512],
                        stg, rec, ALU.mult)

            def proj_chain(mo, bb):
                ps = pps.tile([128, 512], dt.float32, tag="pp",
                              name=f"pp_{mo}_{bb}")
                for kc in range(NM):
                    nc.tensor.matmul(
                        ps,
                        lhsT=wp_sb[:, kc, 128 * mo:128 * mo + 128],
                        rhs=yT_sb[:, kc, 512 * bb:512 * bb + 512],
                        start=(kc == 0), stop=(kc == NM - 1))
                st = stage.tile([128, 512], dt.bfloat16, tag="st",
                                name=f"st_{mo}_{bb}")
                nc.vector.tensor_copy(st, ps)
                nc.sync.dma_start(yp_r[:, mo, 512 * bb:512 * bb + 512], st)

            # Filler queue: QKV chains drip-fed into the attention j-loops so
            # PE never monopolizes long stretches while ScalarE starves.
            work = []
            for bb in range(1, NB):
                work += [partial(qk_chain, "q", 0, bb),
                         partial(qk_chain, "k", 0, bb)]
                work += [partial(v_chain, tt) for tt in range(4 * bb, 4 * bb + 4)]
            for m in range(1, NM):
                for bb in range(NB):
                    work += [partial(qk_chain, "q", m, bb),
                             partial(qk_chain, "k", m, bb)]
            wq_i = iter(work)

            def take(n):
                out = []
                for _ in range(n):
                    f = next(wq_i, None)
                    if f is None:
                        break
                    out.append(f)
                return out

            # prelude: minimum needed for attn(0,0)
            qk_chain("q", 0, 0)
            qk_chain("k", 0, 0)
            for tt in range(4):
                v_chain(tt)

            # pair 0 absorbs its own chains + pair-1's qk (required order);
            # pair-2/3 qk chains spread thin over pairs 1-2 to feed PE there.
            attn_block(0, 0, take(4))
            attn_block(0, 1, take(6))
            attn_block(0, 2, take(8))
            attn_block(0, 3, take(8))
            attn_block(1, 0, take(1))
            attn_block(1, 1, take(2))
            attn_block(1, 2, take(2))
            attn_block(1, 3, take(3))
            attn_block(2, 0, take(2))
            attn_block(2, 1, take(2))
            attn_block(2, 2, take(2))
            attn_block(2, 3, take(2))
            for f in wq_i:  # any remaining QKV chains
                f()

            # QKV psum pool -> projection psum pool (all QKV chains emitted)
            qkps_cm.__exit__(None, None, None)
            pps_cm = tc.tile_pool(name="pps", bufs=2, space="PSUM")
            pps = pps_cm.__enter__()

            # pair 3: largest block first so every proj batch except the last
            # can hide inside a later attention block.
            attn_block(3, 3)
            attn_block(3, 0, [partial(proj_chain, mo, 3)
                              for mo in range(N_EMBD // 128)])
            attn_block(3, 1, [partial(proj_chain, mo, 0)
                              for mo in range(N_EMBD // 128)])
            attn_block(3, 2, [partial(proj_chain, mo, 1)
                              for mo in range(N_EMBD // 128)])
            for mo in range(N_EMBD // 128):
                proj_chain(mo, 2)

            pps_cm.__exit__(None, None, None)
            avps_cm.__exit__(None, None, None)
            sps_cm.__exit__(None, None, None)

    _strip_self_waits(nc, mybir)
    nc.finalize()
    return nc


def _get_nc():
    if "nc" not in _BUILT:
        _BUILT["nc"] = _build_nc()
    return _BUILT["nc"]


def _make_masks():
    sp = np.arange(128)[:, None]
    tp = np.arange(512)[None, :]
    return np.stack([(tp >= 128 * jpos + sp) for jpos in range(4)],
                    axis=1).astype(BF16)  # [128, 4, 512]


def kernel(x, Wq, bq, Wk, bk, Wv, bv, Wp, bp):
    global LAST_RESULT
    from concourse.bass_utils import run_bass_kernel_spmd

    x = np.asarray(x, F32)
    Wq = np.asarray(Wq, F32); bq = np.asarray(bq, F32)
    Wk = np.asarray(Wk, F32); bk = np.asarray(bk, F32)
    Wv = np.asarray(Wv, F32); bv = np.asarray(bv, F32)
    Wp = np.asarray(Wp, F32); bp = np.asarray(bp, F32)

    nc = _get_nc()
    masks = _make_masks()
    xT = np.ascontiguousarray(x.transpose(0, 2, 1))  # [B, C, T]
    # [C, T] -> [NB, 128, NKC, 512]: (kc*128+p, bb*512+t') -> [bb, p, kc, t']
    xT_r = [np.ascontiguousarray(
        xT[b].reshape(NKC, 128, NB, 512).transpose(2, 1, 0, 3)).astype(BF16)
        for b in range(B)]

    def warr(w):  # [1024, 512] -> [128, NKC, 512]
        return np.ascontiguousarray(
            w.reshape(NKC, 128, CPC).transpose(1, 0, 2)).astype(BF16)

    def warr_m(w):  # [1024, 512] -> [NM, 128, NKC, 128] (m-major)
        return np.ascontiguousarray(
            w.reshape(NKC, 128, NM, 128).transpose(2, 1, 0, 3)).astype(BF16)

    in_maps = []
    for c in range(N_CORES):
        b, g = divmod(c, 2)
        sl = slice(CPC * g, CPC * g + CPC)
        in_maps.append({
            "xT": xT_r[b],
            "wq": warr_m(Wq[:, sl]),
            "wk": warr_m(Wk[:, sl]),
            "wv": warr(Wv[:, sl]),
            "wp": np.ascontiguousarray(
                Wp[sl, :].reshape(NM, 128, N_EMBD).transpose(1, 0, 2)).astype(BF16),
            "bq_r": np.ascontiguousarray(bq[sl].reshape(NM, 128).T),
            "bk_r": np.ascontiguousarray(bk[sl].reshape(NM, 128).T),
            "bv_bc": np.ascontiguousarray(
                np.broadcast_to(bv[sl], (128, CPC))).astype(F32),
            "masks": masks,
        })

    res = run_bass_kernel_spmd(nc, in_maps, core_ids=list(range(N_CORES)))
    LAST_RESULT = res

    y = np.empty((B, T, N_EMBD), F32)
    k = np.empty((B, N_HEAD, T, HEAD_DIM), F32)
    v = np.empty((B, N_HEAD, T, HEAD_DIM), F32)
    for c in range(N_CORES):
        b, g = divmod(c, 2)
        out = res.results[c]
        kT = out["kT_out"].astype(F32)           # [512, T]
        vn = out["v_out"].astype(F32)            # [T, 512]
        for lh in range(HPC):
            h = HPC * g + lh
            k[b, h] = kT[64 * lh:64 * lh + 64, :].T
            v[b, h] = vn[:, 64 * lh:64 * lh + 64]
    for b in range(B):
        ypT = (res.results[2 * b]["ypT_out"].astype(F32)
               + res.results[2 * b + 1]["ypT_out"].astype(F32))  # [C, T]
        y[b] = ypT.T + bp[None, :]

    present = np.stack([k, v])  # [2, B, H, T, D]
    return y, present


# revision 29
# speedup vs baseline: 1.2312x; 1.0265x over previous
"""Causal self-attention (B=4, T=2048, C=1024, H=16, D=64) on 8 TRN2 NeuronCores.

Sharding: core c -> (batch b = c//2, head-group g = c%2 covering heads
8g..8g+8). Data-parallel over B, tensor-parallel over heads. The output
projection is computed per-core over its 512 channels; the two partial
products per batch are summed on the host (the "all-reduce"), where the
projection bias is also added.

Per-core kernel (single SPMD program, per-core data):
  - qT = (x Wq + bq)^T and kT likewise, laid out [c'=512, T] (head-major on
    partitions: chunk m holds heads 2m, 2m+1 at partition offsets 0/64); v in
    natural layout [T, c'] with an appended ones column per head (Vaug, M=65)
    for the softmax denominator.
  - attention per head pair m, per 512-wide t-block: S^T[s,t] tiles via K=64
    matmuls packed two-heads-per-PE-array (tile_position derives from
    base_partition 0/64); exp on ScalarE (scale=1/8 folded) straight from PSUM
    into bf16 SBUF; causal masking by multiplying constant triangular masks on
    the diagonal s-chunks; AV matmuls with lhsT=Vaug -> unnormalized y^T (rows
    0:64) and sumexp (row 64) in one PSUM accumulation; normalize with
    partition_broadcast + fast reciprocal.
  - y_out^T = Wp_g^T @ yT accumulated over the core's 4 channel chunks,
    streamed to DRAM as bf16.

QKV compute, attention, and the projection are interleaved so the ScalarE
(exp) stream starts early and PE work hides under it. After Tile scheduling,
redundant same-engine semaphore waits are stripped so cross-engine waits ride
the instructions themselves instead of spawning EVENT_SEMAPHORE ops on the
busy Scalar queue.

No max-subtraction in softmax: scores are O(1) here (exp is safe in fp32),
and exp(S)/sum(exp(S)) is mathematically identical to jax.nn.softmax.
"""

from functools import partial

import numpy as np
import ml_dtypes

BF16 = ml_dtypes.bfloat16
F32 = np.float32

N_EMBD = 1024
N_HEAD = 16
HEAD_DIM = 64
B = 4
T = 2048
N_CORES = 8
HPC = 8          # heads per core
CPC = HPC * HEAD_DIM  # channels per core = 512
NKC = N_EMBD // 128   # contraction chunks over full embed = 8
NM = CPC // 128       # head-pair chunks per core = 4
NB = T // 512         # 512-wide t blocks = 4
NTT = T // 128        # 128-wide t tiles = 16

_BUILT = {}
LAST_RESULT = None  # BassKernelResults of the most recent run (for test harness)


def _strip_self_waits(nc, mybir):
    """Remove same-engine semaphore waits (vacuous on an in-order queue for
    equal-shape streaming ops) so the single HW wait slot is free for the
    real cross-engine dependency."""
    pfx = {
        mybir.EngineType.Activation: "Activation_",
        mybir.EngineType.DVE: "DVE_",
        mybir.EngineType.PE: "PE_",
        mybir.EngineType.Pool: "Pool_",
        mybir.EngineType.SP: "SP_",
    }
    n = 0
    for blk in nc.main_func.blocks:
        for ins in blk.instructions:
            si = ins.sync_info
            if si is None or not si.on_wait:
                continue
            p = pfx.get(ins.engine)
            if not p:
                continue
            kept = [w for w in si.on_wait
                    if not (w.ant_name and w.ant_name.startswith(p)
                            and w.wait_mode == "sem-ge-imm")]
            if len(kept) != len(si.on_wait):
                n += len(si.on_wait) - len(kept)
                si.on_wait = kept
    return n


def _build_nc():
    import concourse.bass as bass
    import concourse.mybir as mybir
    import concourse.tile as tile
    from concourse import bacc

    dt = mybir.dt
    AF = mybir.ActivationFunctionType
    ALU = mybir.AluOpType

    nc = bacc.Bacc(trn_type="TRN2", name="csa")

    # ---- DRAM I/O (host pre-arranges for contiguous per-partition DMA) ----
    xT_d = nc.dram_tensor("xT", [NB, 128, NKC, 512], dt.bfloat16,
                          kind="ExternalInput")
    wq_d = nc.dram_tensor("wq", [NM, 128, NKC, 128], dt.bfloat16,
                          kind="ExternalInput")
    wk_d = nc.dram_tensor("wk", [NM, 128, NKC, 128], dt.bfloat16,
                          kind="ExternalInput")
    wv_d = nc.dram_tensor("wv", [128, NKC, CPC], dt.bfloat16, kind="ExternalInput")
    wp_d = nc.dram_tensor("wp", [128, NM, N_EMBD], dt.bfloat16,
                          kind="ExternalInput")
    bq_d = nc.dram_tensor("bq_r", [128, NM], dt.float32, kind="ExternalInput")
    bk_d = nc.dram_tensor("bk_r", [128, NM], dt.float32, kind="ExternalInput")
    bv_d = nc.dram_tensor("bv_bc", [128, CPC], dt.float32, kind="ExternalInput")
    mk_d = nc.dram_tensor("masks", [128, 4, 512], dt.bfloat16, kind="ExternalInput")

    kT_o = nc.dram_tensor("kT_out", [CPC, T], dt.bfloat16, kind="ExternalOutput")
    v_o = nc.dram_tensor("v_out", [T, CPC], dt.bfloat16, kind="ExternalOutput")
    yp_o = nc.dram_tensor("ypT_out", [N_EMBD, T], dt.bfloat16, kind="ExternalOutput")

    with tile.TileContext(nc) as tc:
        with tc.tile_pool(name="const", bufs=1) as cp, \
             tc.tile_pool(name="ex", bufs=6) as exp_pool, \
             tc.tile_pool(name="stage", bufs=4) as stage, \
             tc.tile_pool(name="small", bufs=4) as small:

            # ---- load inputs, ordered so compute can start early ----
            # m-major weight tiles so the first chain's weights arrive fast
            wq_sb = cp.tile([128, NM, NKC, 128], dt.bfloat16, tag="wq")
            nc.sync.dma_start(wq_sb[:, 0], wq_d.ap()[0])
            # x^T, bb-major: [p, bb, kc, 512]
            xT_sb = cp.tile([128, NB, NKC, 512], dt.bfloat16, tag="xT")
            nc.sync.dma_start(xT_sb[:, 0], xT_d.ap()[0])
            wk_sb = cp.tile([128, NM, NKC, 128], dt.bfloat16, tag="wk")
            nc.sync.dma_start(wk_sb[:, 0], wk_d.ap()[0])
            # remaining loads split across the vector HWDGE queue so the sync
            # queue serves the startup-critical xT/W stream unimpeded
            bq_sb = cp.tile([128, NM], dt.float32, tag="bq")
            nc.scalar.dma_start(bq_sb, bq_d.ap())
            bk_sb = cp.tile([128, NM], dt.float32, tag="bk")
            nc.scalar.dma_start(bk_sb, bk_d.ap())
            wv_sb = cp.tile([128, NKC, CPC], dt.bfloat16, tag="wv")
            nc.scalar.dma_start(wv_sb, wv_d.ap())
            bv_sb = cp.tile([128, CPC], dt.float32, tag="bv")
            nc.scalar.dma_start(bv_sb, bv_d.ap())
            mk_sb = cp.tile([128, 4, 512], dt.bfloat16, tag="mk")
            nc.scalar.dma_start(mk_sb, mk_d.ap())
            wp_sb = cp.tile([128, NM, N_EMBD], dt.bfloat16, tag="wp")
            nc.scalar.dma_start(wp_sb, wp_d.ap())
            for bb in range(1, NB):
                nc.sync.dma_start(xT_sb[:, bb], xT_d.ap()[bb])
            for m in range(1, NM):
                nc.sync.dma_start(wq_sb[:, m], wq_d.ap()[m])
                nc.sync.dma_start(wk_sb[:, m], wk_d.ap()[m])

            qT_sb = cp.tile([128, NM, T], dt.bfloat16, tag="qT")
            kT_sb = cp.tile([128, NM, T], dt.bfloat16, tag="kT")
            # Vaug: [t%128, tt, head, 65] (col 64 = ones)
            v_sb = cp.tile([128, NTT, HPC, 65], dt.bfloat16, tag="v")
            yT_sb = cp.tile([128, NM, T], dt.bfloat16, tag="yT")

            nc.gpsimd.memset(v_sb[:, :, :, 64:65], 1.0)

            kT_o_r = kT_o.ap().rearrange("(m p) t -> p m t", p=128)
            v_o_r = v_o.ap().rearrange("(tt p) (h d) -> p tt h d", p=128, h=HPC)
            yp_r = yp_o.ap().rearrange("(mo p) t -> p mo t", p=128)

            # PSUM pools: sps + avps live throughout; qkps (QKV) is swapped
            # for pps (projection) once the last QK pair is emitted.
            sps_cm = tc.tile_pool(name="sps", bufs=2, space="PSUM")
            sps = sps_cm.__enter__()
            avps_cm = tc.tile_pool(name="avps", bufs=1, space="PSUM")
            avps = avps_cm.__enter__()
            qkps_cm = tc.tile_pool(name="qkps", bufs=2, space="PSUM")
            qkps = qkps_cm.__enter__()

            def qk_chain(which, m, bb):
                w_sb, bias_sb, out_sb = ((wq_sb, bq_sb, qT_sb) if which == "q"
                                         else (wk_sb, bk_sb, kT_sb))
                ps = qkps.tile([128, 512], dt.float32, tag="qk",
                               name=f"{which}_{m}_{bb}")
                for kc in range(NKC):
                    nc.tensor.matmul(
                        ps,
                        lhsT=w_sb[:, m, kc, :],
                        rhs=xT_sb[:, bb, kc, :],
                        start=(kc == 0), stop=(kc == NKC - 1))
                nc.vector.tensor_scalar(
                    out=out_sb[:, m, 512 * bb:512 * bb + 512],
                    in0=ps, scalar1=bias_sb[:, m:m + 1], scalar2=None,
                    op0=ALU.add)
                if which == "k":
                    nc.sync.dma_start(kT_o_r[:, m, 512 * bb:512 * bb + 512],
                                      kT_sb[:, m, 512 * bb:512 * bb + 512])

            def v_chain(tt):
                ps = qkps.tile([128, 512], dt.float32, tag="qk", name=f"v_{tt}")
                for kc in range(NKC):
                    nc.tensor.matmul(
                        ps,
                        lhsT=xT_sb[:, tt // 4, kc, 128 * (tt % 4):128 * (tt % 4) + 128],
                        rhs=wv_sb[:, kc, :],
                        start=(kc == 0), stop=(kc == NKC - 1))
                nc.vector.tensor_tensor(
                    out=v_sb[:, tt, :, 0:64],
                    in0=ps.rearrange("p (h d) -> p h d", h=HPC),
                    in1=bv_sb.rearrange("p (h d) -> p h d", h=HPC),
                    op=ALU.add)
                nc.sync.dma_start(v_o_r[:, tt], v_sb[:, tt, :, 0:64])

            def attn_block(m, bb, fillers=()):
                njc = 4 * bb + 4  # causal s-chunk count for this t block
                ps_y = [avps.tile([65, 512], dt.float32, tag=f"av{h}",
                                  name=f"av_{h}_{m}_{bb}")
                        for h in range(2)]
                fit = iter(fillers)
                for j in range(njc):
                    f = next(fit, None)
                    if f is not None:
                        f()
                    jpos = j - 4 * bb
                    toff = 128 * jpos if jpos > 0 else 0
                    W = 512 - toff  # causally-valid width of this chunk
                    # one PSUM tile holds BOTH heads' S^T for chunk j; the two
                    # K=64 row-tiled matmuls (tile_position rows 0/64) become
                    # ready together (single exp releases the slot), stay
                    # adjacent in the PE stream, and co-issue.
                    ps_s = sps.tile([128, 2, 512], dt.float32, tag="s",
                                    name=f"s_{m}_{bb}_{j}")
                    for half in range(2):
                        p0 = 64 * half
                        nc.tensor.matmul(
                            ps_s[:, half, 0:W],
                            lhsT=kT_sb[p0:p0 + 64, m, 128 * j:128 * j + 128],
                            rhs=qT_sb[p0:p0 + 64, m,
                                      512 * bb + toff:512 * bb + 512],
                            start=True, stop=True)
                    ex = exp_pool.tile([128, 2, 512], dt.bfloat16,
                                       tag="ex", name=f"ex_{m}_{bb}_{j}")
                    nc.scalar.activation(ex[:, :, 0:W], ps_s[:, :, 0:W],
                                         AF.Exp, scale=0.125)
                    if jpos >= 0:  # diagonal chunk: causal mask (both heads)
                        nc.vector.tensor_tensor(
                            ex[:, :, 0:W], ex[:, :, 0:W],
                            mk_sb[:, jpos:jpos + 1, toff:512].to_broadcast(
                                [128, 2, W]),
                            ALU.mult)
                    for half in range(2):
                        nc.tensor.matmul(
                            ps_y[half][:, toff:512],
                            lhsT=v_sb[:, j, 2 * m + half, :],
                            rhs=ex[:, half, 0:W],
                            start=(j == 0), stop=(j == njc - 1),
                            skip_group_check=True)
                for f in fit:  # any leftover fillers
                    f()
                # stage all PSUM reads first so both AV slots free quickly
                rows, stgs = [], []
                for half in range(2):
                    row = small.tile([1, 512], dt.float32, tag="row",
                                     name=f"row_{m}_{bb}_{half}")
                    nc.vector.tensor_copy(row, ps_y[half][64:65, :])
                    stg = small.tile([64, 512], dt.float32, tag="stg",
                                     name=f"stg_{m}_{bb}_{half}")
                    nc.vector.tensor_copy(stg, ps_y[half][0:64, :])
                    rows.append(row)
                    stgs.append(stg)
                for half in range(2):
                    bc = small.tile([64, 512], dt.float32, tag="bc",
                                    name=f"bc_{m}_{bb}_{half}")
                    nc.gpsimd.partition_broadcast(bc, rows[half], channels=64)
                    rec = small.tile([64, 512], dt.float32, tag="rec",
                                     name=f"rec_{m}_{bb}_{half}")
                    nc.vector.reciprocal_approx_fast(out=rec, in_=bc)
                    nc.vector.tensor_tensor(
                        yT_sb[64 * half:64 * half + 64, m,
                              512 * bb:512 * bb + 512],
                        stgs[half], rec, ALU.mult)

            def proj_chain(mo, bb):
                ps = pps.tile([128, 512], dt.float32, tag="pp",
                              name=f"pp_{mo}_{bb}")
                for kc in range(NM):
                    nc.tensor.matmul(
                        ps,
                        lhsT=wp_sb[:, kc, 128 * mo:128 * mo + 128],
                        rhs=yT_sb[:, kc, 512 * bb:512 * bb + 512],
                        start=(kc == 0), stop=(kc == NM - 1))
                st = stage.tile([128, 512], dt.bfloat16, tag="st",
                                name=f"st_{mo}_{bb}")
                nc.vector.tensor_copy(st, ps)
                nc.sync.dma_start(yp_r[:, mo, 512 * bb:512 * bb + 512], st)

            # Filler queue: QKV chains drip-fed into the attention j-loops so
            # PE never monopolizes long stretches while ScalarE starves.
            work = []
            for bb in range(1, NB):
                work += [partial(qk_chain, "q", 0, bb),
                         partial(qk_chain, "k", 0, bb)]
                work += [partial(v_chain, tt) for tt in range(4 * bb, 4 * bb + 4)]
            for m in range(1, NM):
                for bb in range(NB):
                    work += [partial(qk_chain, "q", m, bb),
                             partial(qk_chain, "k", m, bb)]
            wq_i = iter(work)

            def take(n):
                out = []
                for _ in range(n):
                    f = next(wq_i, None)
                    if f is None:
                        break
                    out.append(f)
                return out

            # prelude: minimum needed for attn(0,0)
            qk_chain("q", 0, 0)
            qk_chain("k", 0, 0)
            for tt in range(4):
                v_chain(tt)

            # pair 0 absorbs its own chains + pair-1's qk (required order);
            # pair-2/3 qk chains spread thin over pairs 1-2 to feed PE there.
            attn_block(0, 0, take(4))
            attn_block(0, 1, take(6))
            attn_block(0, 2, take(8))
            attn_block(0, 3, take(8))
            attn_block(1, 0, take(1))
            attn_block(1, 1, take(2))
            attn_block(1, 2, take(2))
            attn_block(1, 3, take(3))
            attn_block(2, 0, take(2))
            attn_block(2, 1, take(2))
            attn_block(2, 2, take(2))
            attn_block(2, 3, take(2))
            for f in wq_i:  # any remaining QKV chains
                f()

            # QKV psum pool -> projection psum pool (all QKV chains emitted)
            qkps_cm.__exit__(None, None, None)
            pps_cm = tc.tile_pool(name="pps", bufs=2, space="PSUM")
            pps = pps_cm.__enter__()

            # pair 3: largest block first so every proj batch except the last
            # can hide inside a later attention block.
            attn_block(3, 3)
            attn_block(3, 0, [partial(proj_chain, mo, 3)
                              for mo in range(N_EMBD // 128)])
            attn_block(3, 1, [partial(proj_chain, mo, 0)
                              for mo in range(N_EMBD // 128)])
            attn_block(3, 2, [partial(proj_chain, mo, 1)
                              for mo in range(N_EMBD // 128)])
            for mo in range(N_EMBD // 128):
                proj_chain(mo, 2)

            pps_cm.__exit__(None, None, None)
            avps_cm.__exit__(None, None, None)
            sps_cm.__exit__(None, None, None)

    _strip_self_waits(nc, mybir)
    nc.finalize()
    return nc


def _get_nc():
    if "nc" not in _BUILT:
        _BUILT["nc"] = _build_nc()
    return _BUILT["nc"]


def _make_masks():
    sp = np.arange(128)[:, None]
    tp = np.arange(512)[None, :]
    return np.stack([(tp >= 128 * jpos + sp) for jpos in range(4)],
                    axis=1).astype(BF16)  # [128, 4, 512]


def kernel(x, Wq, bq, Wk, bk, Wv, bv, Wp, bp):
    global LAST_RESULT
    from concourse.bass_utils import run_bass_kernel_spmd

    x = np.asarray(x, F32)
    Wq = np.asarray(Wq, F32); bq = np.asarray(bq, F32)
    Wk = np.asarray(Wk, F32); bk = np.asarray(bk, F32)
    Wv = np.asarray(Wv, F32); bv = np.asarray(bv, F32)
    Wp = np.asarray(Wp, F32); bp = np.asarray(bp, F32)

    nc = _get_nc()
    masks = _make_masks()
    xT = np.ascontiguousarray(x.transpose(0, 2, 1))  # [B, C, T]
    # [C, T] -> [NB, 128, NKC, 512]: (kc*128+p, bb*512+t') -> [bb, p, kc, t']
    xT_r = [np.ascontiguousarray(
        xT[b].reshape(NKC, 128, NB, 512).transpose(2, 1, 0, 3)).astype(BF16)
        for b in range(B)]

    def warr(w):  # [1024, 512] -> [128, NKC, 512]
        return np.ascontiguousarray(
            w.reshape(NKC, 128, CPC).transpose(1, 0, 2)).astype(BF16)

    def warr_m(w):  # [1024, 512] -> [NM, 128, NKC, 128] (m-major)
        return np.ascontiguousarray(
            w.reshape(NKC, 128, NM, 128).transpose(2, 1, 0, 3)).astype(BF16)

    in_maps = []
    for c in range(N_CORES):
        b, g = divmod(c, 2)
        sl = slice(CPC * g, CPC * g + CPC)
        in_maps.append({
            "xT": xT_r[b],
            "wq": warr_m(Wq[:, sl]),
            "wk": warr_m(Wk[:, sl]),
            "wv": warr(Wv[:, sl]),
            "wp": np.ascontiguousarray(
                Wp[sl, :].reshape(NM, 128, N_EMBD).transpose(1, 0, 2)).astype(BF16),
            "bq_r": np.ascontiguousarray(bq[sl].reshape(NM, 128).T),
            "bk_r": np.ascontiguousarray(bk[sl].reshape(NM, 128).T),
            "bv_bc": np.ascontiguousarray(
                np.broadcast_to(bv[sl], (128, CPC))).astype(F32),
            "masks": masks,
        })

    res = run_bass_kernel_spmd(nc, in_maps, core_ids=list(range(N_CORES)))
    LAST_RESULT = res

    y = np.empty((B, T, N_EMBD), F32)
    k = np.empty((B, N_HEAD, T, HEAD_DIM), F32)
    v = np.empty((B, N_HEAD, T, HEAD_DIM), F32)
    for c in range(N_CORES):
        b, g = divmod(c, 2)
        out = res.results[c]
        kT = out["kT_out"].astype(F32)           # [512, T]
        vn = out["v_out"].astype(F32)            # [T, 512]
        for lh in range(HPC):
            h = HPC * g + lh
            k[b, h] = kT[64 * lh:64 * lh + 64, :].T
            v[b, h] = vn[:, 64 * lh:64 * lh + 64]
    for b in range(B):
        ypT = (res.results[2 * b]["ypT_out"].astype(F32)
               + res.results[2 * b + 1]["ypT_out"].astype(F32))  # [C, T]
        y[b] = ypT.T + bp[None, :]

    present = np.stack([k, v])  # [2, B, H, T, D]
    return y, present
